# revision 30
# baseline (speedup 1.0000x reference)
# Trainium2 Bass kernel for nn_CustomAttention (cosine-sim multi-head attention).
#
# Sharding over 8 cores: core c handles batch b = c//2 and head group
# g = c%2 (8 of 16 heads, 512 feature dims).  Each core computes its heads'
# q/k/v projections (Megatron column-parallel), cosine-sim attention, and a
# partial output projection (row-parallel over its 512 dims).  The host sums
# the two partial outputs per batch and adds out_b.
#
# All heavy matmuls run in fp16 (1 cycle/row on the PE at any output width),
# with fp32 PSUM accumulation.  Layout highlights:
#   qT/kT: (dims=512, seq=1024) as 4 tiles of (128, 1024); head dims on
#          partitions so the scores matmul contracts head_dim on partitions.
#   scores are computed transposed, sT[k, q]; the per-key ls/||k|| factor and
#   the -ls bias fold into the exp() activation as per-partition APs.
#   1/||q|| and ls/||k|| come from exp(-0.5*ln(ssq) [+ ln ls]) so the whole
#   kernel uses a single activation table (ln+exp) -- no table reloads.
#   p@v runs transposed: x[q, d] = eT[k, q-block]^T @ v[k, d|1]; the appended
#   ones-column of v gives the softmax denominator, applied as a per-partition
#   (per-query) scalar during the PSUM->SBUF copy.  x tiles are PE-transposed
#   back to (dims, seq) for the out-proj.
#   out-proj is split into two half-contractions: t0+t1 overlaps the attention
#   stream, t2+t3 runs at the tail (the A half folded into the PSUM
#   accumulation via an identity matmul; copy-out split between DVE and ACT).
#
# Scheduling: engine queues drain strictly in issue order, so issue order ==
# execution order.  The attention main loop is ACT-paced (64 wide exps); the
# PE stream weaves scores, p@v, the v projection, the k projection for tiles
# t1..t3 (only q and k-t0 are done up front), x transposes and the first
# out-proj half into the gaps as "fillers".  GPSIMD cannot touch PSUM, so all
# PSUM->SBUF traffic is on DVE (and ACT Copy in the tail).

import math
import sys
from collections import deque

import numpy as np

sys.path.insert(0, "/opt/trn_rl_repo")

import concourse.bass as bass
import concourse.tile as tile
from concourse import bacc, mybir
from concourse.bass_utils import run_bass_kernel_spmd
from concourse.hw_specs import get_activation_tables

N = 1024  # sequence length
B = 4  # batch
C = 1024  # channels
H = 16  # total heads
HD = 64  # head dim
G = 512  # dims per core (8 heads)
NT = 4  # (128, N) tiles of qT/kT per core
CC = 8  # contraction chunks of 128 over C
ST = 8  # seq tiles of 128
QC = 2  # seq chunks of 512
LOGIT_SCALE_MAX = math.log(1.0 / 0.01)

F32 = mybir.dt.float32
F32R = mybir.dt.float32r
F16 = mybir.dt.float16
AF = mybir.ActivationFunctionType

_CACHED_NC = {}
_LAST_IN_MAPS = None


def build_nc(vbias_nonzero):
    nc = bacc.Bacc("TRN2", target_bir_lowering=False)

    qt_d = nc.declare_dram_parameter("qt", [C, N], F16, isOutput=False)
    kt_d = nc.declare_dram_parameter("kt", [C, N], F16, isOutput=False)
    vt_d = nc.declare_dram_parameter("vt", [C, N], F16, isOutput=False)
    wq_d = nc.declare_dram_parameter("wq", [C, G], F16, isOutput=False)
    wk_d = nc.declare_dram_parameter("wk", [C, G], F16, isOutput=False)
    wv_d = nc.declare_dram_parameter("wv", [C, G], F16, isOutput=False)
    wo_d = nc.declare_dram_parameter("wo", [G, C], F16, isOutput=False)
    bq_d = nc.declare_dram_parameter("bq", [128, NT], F32, isOutput=False)
    bk_d = nc.declare_dram_parameter("bk", [128, NT], F32, isOutput=False)
    bv_d = nc.declare_dram_parameter("bv", [1, G], F16, isOutput=False)
    sel8_d = nc.declare_dram_parameter("sel8", [NT, 128, 8], F16, isOutput=False)
    sel2_d = nc.declare_dram_parameter("sel2", [128, 2], F16, isOutput=False)
    sel8T_d = nc.declare_dram_parameter("sel8T", [8, NT, 128], F32R, isOutput=False)
    lnls_d = nc.declare_dram_parameter("lnls", [2, NT], F32, isOutput=False)
    lsbias_d = nc.declare_dram_parameter("lsbias", [128, 8], F32, isOutput=False)
    ident_d = nc.declare_dram_parameter("ident", [128, 128], F32R, isOutput=False)
    ones1_d = nc.declare_dram_parameter("ones1", [1, 128], F16, isOutput=False)
    out_d = nc.declare_dram_parameter("out", [N, C], F16, isOutput=True)

    qt_r = qt_d[:].rearrange("(cc p) n -> cc p n", p=128)
    kt_r = kt_d[:].rearrange("(cc p) n -> cc p n", p=128)
    vt_r = vt_d[:].rearrange("(cc p) n -> cc p n", p=128)
    wq_r = wq_d[:].rearrange("(g cc p) o -> g p cc o", g=2, p=128)
    wk_r = wk_d[:].rearrange("(g cc p) o -> g p cc o", g=2, p=128)
    wv_r = wv_d[:].rearrange("(g cc p) o -> g p cc o", g=2, p=128)

    # pre-load the ln+exp activation table once; every ACT op in this kernel
    # (Ln, Exp, Copy) is servable from it, so the auto-inserted loads (which
    # thrash between exp-only and ln-only tables) are avoided.
    table_names = list(get_activation_tables(nc.m.arch).keys())
    lnexp_id = table_names.index("natural_log_exp_and_others")

    with tile.TileContext(nc) as tc:
        nc.scalar.add_instruction(
            mybir.InstLoadActFuncSet(
                name=nc.get_next_instruction_name(), ins=[], outs=[],
                act_func_set_id=lnexp_id,
            )
        )
        with (
            tc.tile_pool(name="consts", bufs=1) as consts,
            tc.tile_pool(name="wo_p", bufs=1) as wo_p,
            tc.tile_pool(name="w_p", bufs=3) as w_p,
            tc.tile_pool(name="acts", bufs=16) as acts,
            tc.tile_pool(name="big", bufs=1) as big,
            tc.tile_pool(name="sq_p", bufs=2) as sq_p,
            tc.tile_pool(name="stats", bufs=1) as stats,
            tc.tile_pool(name="lssq_p", bufs=2) as lssq_p,
            tc.tile_pool(name="eT_p", bufs=26) as eT_p,
            tc.tile_pool(name="x_p", bufs=3) as x_p,
            tc.tile_pool(name="den_p", bufs=2) as den_p,
            tc.tile_pool(name="oB_p", bufs=4) as oB_p,
        ):
            # ---- persistent tiles ----
            qT = [big.tile([128, N], F16, tag=f"qT{t}", name=f"qT{t}") for t in range(NT)]
            kT = [big.tile([128, N], F16, tag=f"kT{t}", name=f"kT{t}") for t in range(NT)]
            v_sb = [big.tile([128, 8, HD + 1], F16, tag=f"v{s}", name=f"v{s}") for s in range(ST)]
            xt = [big.tile([128, N], F16, tag=f"xt{t}", name=f"xt{t}") for t in range(NT)]
            rskT = stats.tile([128, ST, 8], F32)
            rsq = stats.tile([8, N], F32R)
            rsk_t = [
                stats.tile([2, N], F32R, tag=f"rsk{t}", name=f"rsk{t}")
                for t in range(NT)
            ]
            # ones column of v (softmax denominator); SBUF-only so GPSIMD ok.
            for s in range(ST):
                nc.gpsimd.memset(v_sb[s][:, :, HD], 1.0)

            # ---- DMA stream (single SP queue; issue order = transfer order) ----
            # q stream first: the first projection wave starts ~2us in.
            wq_sb = w_p.tile([128, CC, G], F16, tag="w", name="wq")
            qch = []
            a0 = acts.tile([128, N], F16, tag="act", name="qt0")
            nc.sync.dma_start(out=a0[:], in_=qt_r[0])
            qch.append(a0)
            nc.sync.dma_start(out=wq_sb[:, 0:4, :], in_=wq_r[0])
            for cc in range(1, 4):
                a = acts.tile([128, N], F16, tag="act", name=f"qt{cc}")
                nc.sync.dma_start(out=a[:], in_=qt_r[cc])
                qch.append(a)
            nc.sync.dma_start(out=wq_sb[:, 4:8, :], in_=wq_r[1])
            for cc in range(4, 8):
                a = acts.tile([128, N], F16, tag="act", name=f"qt{cc}")
                nc.sync.dma_start(out=a[:], in_=qt_r[cc])
                qch.append(a)

            # consts needed during the q projection
            sel8 = consts.tile([128, NT, 8], F16)
            nc.sync.dma_start(out=sel8[:], in_=sel8_d[:].rearrange("t p e -> p t e"))
            sel2 = consts.tile([128, 2], F16)
            nc.sync.dma_start(out=sel2[:], in_=sel2_d[:])
            bq_sb = consts.tile([128, NT], F32)
            nc.sync.dma_start(out=bq_sb[:], in_=bq_d[:])

            wk_sb = w_p.tile([128, CC, G], F16, tag="w", name="wk")
            nc.sync.dma_start(out=wk_sb[:, 0:4, :], in_=wk_r[0])
            nc.sync.dma_start(out=wk_sb[:, 4:8, :], in_=wk_r[1])
            kch = []
            for cc in range(CC):
                a = acts.tile([128, N], F16, tag="act", name=f"kt{cc}")
                nc.sync.dma_start(out=a[:], in_=kt_r[cc])
                kch.append(a)

            sel8T = consts.tile([8, NT, 128], F32R)
            nc.sync.dma_start(out=sel8T[:], in_=sel8T_d[:])
            bk_sb = consts.tile([128, NT], F32)
            nc.sync.dma_start(out=bk_sb[:], in_=bk_d[:])
            lnls = consts.tile([2, NT], F32)
            nc.sync.dma_start(out=lnls[:], in_=lnls_d[:])
            ident = consts.tile([128, 128], F32R)
            nc.sync.dma_start(out=ident[:], in_=ident_d[:])
            lsbias = consts.tile([128, 8], F32)
            nc.sync.dma_start(out=lsbias[:], in_=lsbias_d[:])

            wv_sb = w_p.tile([128, CC, G], F16, tag="w", name="wv")
            nc.sync.dma_start(out=wv_sb[:, 0:4, :], in_=wv_r[0])
            nc.sync.dma_start(out=wv_sb[:, 4:8, :], in_=wv_r[1])
            vch = []
            for cc in range(CC):
                a = acts.tile([128, N], F16, tag="act", name=f"vt{cc}")
                nc.sync.dma_start(out=a[:], in_=vt_r[cc])
                vch.append(a)

            ones1 = consts.tile([1, 128], F16)
            nc.sync.dma_start(out=ones1[:], in_=ones1_d[:])
            bv_sb = consts.tile([1, G], F16)
            nc.sync.dma_start(out=bv_sb[:], in_=bv_d[:])
            wo_sb = wo_p.tile([128, NT, C], F16)
            nc.sync.dma_start(
                out=wo_sb[:], in_=wo_d[:].rearrange("(t p) c -> p t c", p=128)
            )

            # ---- phase 1: q projection (full) + k projection tile 0 ----
            deferred = deque()

            def flush(n=99):
                for _ in range(min(n, len(deferred))):
                    deferred.popleft()()

            with (
                tc.tile_pool(name="pp", bufs=4, space="PSUM") as pp,
                tc.tile_pool(name="pssq", bufs=2, space="PSUM") as pssq_p,
                tc.tile_pool(name="pbc", bufs=2, space="PSUM") as pbc,
            ):
                # q projection in 4 pair-tile waves; global per-head ssq.
                def q_wave(qc, tp, ssq_half):
                    pst = [
                        pp.tile([128, G], F32, tag="proj", name=f"qp{qc}{tp}{i}")
                        for i in range(2)
                    ]
                    for cc in range(CC):
                        for ti in range(2):
                            t = 2 * tp + ti
                            nc.tensor.matmul(
                                pst[ti][:],
                                wq_sb[:, cc, t * 128 : (t + 1) * 128],
                                qch[cc][:, qc * 512 : (qc + 1) * 512],
                                start=(cc == 0),
                                stop=(cc == CC - 1),
                            )
                    for ti in range(2):
                        t = 2 * tp + ti
                        nc.vector.tensor_scalar_add(
                            out=qT[t][:, qc * 512 : (qc + 1) * 512],
                            in0=pst[ti][:],
                            scalar1=bq_sb[:, t : t + 1],
                        )
                        sq = sq_p.tile([128, G], F16, tag="sq", name=f"sq{t}{qc}")
                        nc.vector.tensor_mul(
                            out=sq[:],
                            in0=qT[t][:, qc * 512 : (qc + 1) * 512],
                            in1=qT[t][:, qc * 512 : (qc + 1) * 512],
                        )

                        def ssq_mm(t=t, sq=sq):
                            nc.tensor.matmul(
                                ssq_half[:],
                                sel8[:, t, :],
                                sq[:],
                                start=(t == 0),
                                stop=(t == NT - 1),
                            )

                        deferred.append(ssq_mm)

                def q_norm_thunk(ssq_half, qc):
                    def run():
                        lssq = lssq_p.tile([8, G], F32, tag="lssq", name=f"lsq{qc}")
                        nc.scalar.activation(out=lssq[:], in_=ssq_half[:], func=AF.Ln)
                        nc.scalar.activation(
                            out=rsq[:, qc * 512 : (qc + 1) * 512],
                            in_=lssq[:], func=AF.Exp, scale=-0.5,
                        )

                    return run

                def bc_thunk(t, qc):
                    def run():
                        pb = pbc.tile([128, G], F32, tag="bc", name=f"bc{t}{qc}")
                        nc.tensor.matmul(
                            pb[:],
                            sel8T[:, t, :],
                            rsq[:, qc * 512 : (qc + 1) * 512],
                            start=True,
                            stop=True,
                        )
                        nc.vector.tensor_mul(
                            out=qT[t][:, qc * 512 : (qc + 1) * 512],
                            in0=qT[t][:, qc * 512 : (qc + 1) * 512],
                            in1=pb[:],
                        )

                    return run

                for qc in range(QC):
                    ssq_half = pssq_p.tile([8, G], F32, tag="ssq", name=f"sq_q{qc}")
                    for tp in range(2):
                        q_wave(qc, tp, ssq_half)
                        flush(2)
                    flush()
                    deferred.append(q_norm_thunk(ssq_half, qc))
                    for t in range(NT):
                        deferred.append(bc_thunk(t, qc))

                # k projection: single-tile waves.  Tile 0 runs here (prefix);
                # tiles 1..3 run as fillers inside the attention loop using the
                # phase-2 "po" pool.  Per-(tile, half) ssq + norm chains.
                def k_wave_half(t, qc, ps, half):
                    for cc in range(4 * half, 4 * half + 4):
                        nc.tensor.matmul(
                            ps[:],
                            wk_sb[:, cc, t * 128 : (t + 1) * 128],
                            kch[cc][:, qc * 512 : (qc + 1) * 512],
                            start=(cc == 0),
                            stop=(cc == CC - 1),
                        )

                def k_wave_finish(t, qc, ps):
                    nc.vector.tensor_scalar_add(
                        out=kT[t][:, qc * 512 : (qc + 1) * 512],
                        in0=ps[:],
                        scalar1=bk_sb[:, t : t + 1],
                    )
                    sq = sq_p.tile([128, G], F16, tag="sq", name=f"ksq{t}{qc}")
                    nc.vector.tensor_mul(
                        out=sq[:],
                        in0=kT[t][:, qc * 512 : (qc + 1) * 512],
                        in1=kT[t][:, qc * 512 : (qc + 1) * 512],
                    )
                    return sq

                def k_wave(t, qc, pool, ptag):
                    ps = pool.tile([128, G], F32, tag=ptag, name=f"kp{t}{qc}")
                    k_wave_half(t, qc, ps, 0)
                    k_wave_half(t, qc, ps, 1)
                    return k_wave_finish(t, qc, ps)

                def k_chain(t, qc, sq, pool, ptag):
                    # ssq (2 rows) -> sbuf bounce -> ln -> exp into rsk_t[t].
                    # The DVE bounce frees the PSUM tile immediately instead of
                    # holding it until ACT's (busy, in-order) queue reaches Ln.
                    ps = pool.tile([128, G], F32, tag=ptag, name=f"kn{t}{qc}")
                    nc.tensor.matmul(
                        ps[0:2, :], sel2[:], sq[:], start=True, stop=True
                    )
                    ssb = lssq_p.tile([2, G], F32, tag="ssb", name=f"skb{t}{qc}")
                    nc.vector.tensor_copy(out=ssb[:], in_=ps[0:2, :])
                    lssq = lssq_p.tile([2, G], F32, tag="lssq", name=f"lsk{t}{qc}")
                    nc.scalar.activation(out=lssq[:], in_=ssb[:], func=AF.Ln)
                    nc.scalar.activation(
                        out=rsk_t[t][:, qc * 512 : (qc + 1) * 512],
                        in_=lssq[:], func=AF.Exp, scale=-0.5,
                        bias=lnls[:, t : t + 1],
                    )

                def k_rskT(t, qc, pool, ptag):
                    # transpose rsk[2t:2t+2, half] into rskT[:, s, 2t:2t+2]
                    # for the 4 key tiles s of this half, all in one psum tile.
                    pt = pool.tile([128, G], F32, tag=ptag, name=f"krt{t}{qc}")
                    for i in range(4):
                        s = 4 * qc + i
                        nc.tensor.matmul(
                            pt[:].bitcast(F32R)[:, 2 * i : 2 * i + 2],
                            rsk_t[t][:, s * 128 : (s + 1) * 128],
                            ident[0:2, 0:2],
                            is_transpose=True,
                            start=(i == 0),
                            stop=(i == 3),
                        )
                    nc.vector.tensor_copy(
                        out=rskT[:, 4 * qc : 4 * qc + 4, 2 * t : 2 * t + 2],
                        in_=pt[:, 0:8].rearrange("p (s e) -> p s e", e=2),
                    )

                for t in range(2):
                    for qc in range(QC):
                        sq = k_wave(t, qc, pp, "proj")
                        flush(2)
                        k_chain(t, qc, sq, pbc, "bc")
                        k_rskT(t, qc, pbc, "bc")
                flush()

            # ---- phase 2: attention + v proj + k t1..3 + out-projection ----
            with (
                tc.tile_pool(name="psT", bufs=2, space="PSUM") as psT_p,
                tc.tile_pool(name="pv", bufs=1, space="PSUM") as pv_p,
                tc.tile_pool(name="po", bufs=2, space="PSUM") as po_p,
            ):
                ets = {}
                cur_pv = {}
                cur_x = [None] * 8
                vp_issued = [False] * ST
                vp_step = [99] * ST
                cur_g = [0]
                fill_hi = deque()
                fill_lo = deque()

                def sc(h, s):
                    t, j = divmod(h, 2)
                    st = psT_p.tile([128, N], F32, tag="sT", name=f"sT{h}_{s}")
                    for qc in range(QC):
                        nc.tensor.matmul(
                            st[:, qc * 512 : (qc + 1) * 512],
                            kT[t][j * 64 : (j + 1) * 64, s * 128 : (s + 1) * 128],
                            qT[t][j * 64 : (j + 1) * 64, qc * 512 : (qc + 1) * 512],
                            start=True,
                            stop=True,
                        )
                    e = eT_p.tile([128, N], F16, tag="eT", name=f"eT{h}_{s}")
                    nc.scalar.activation(
                        out=e[:], in_=st[:], func=AF.Exp,
                        bias=lsbias[:, h : h + 1],
                        scale=rskT[:, s, h : h + 1],
                    )
                    ets[(h, s)] = e

                vp_state = {}

                def vp_q(s, qtr):
                    if qtr == 0:
                        ps = po_p.tile([128, G], F32, tag="po", name=f"vp{s}")
                        vp_state[s] = ps
                    else:
                        ps = vp_state[s]
                    for cc in range(2 * qtr, 2 * qtr + 2):
                        nc.tensor.matmul(
                            ps[:],
                            vch[cc][:, s * 128 : (s + 1) * 128],
                            wv_sb[:, cc, :],
                            start=(cc == 0),
                            stop=(not vbias_nonzero and cc == CC - 1),
                        )
                    if qtr == 3:
                        del vp_state[s]
                        if vbias_nonzero:
                            nc.tensor.matmul(
                                ps[:], ones1[:], bv_sb[:], start=False, stop=True
                            )
                        nc.vector.tensor_copy(
                            out=v_sb[s][:, :, 0:HD],
                            in_=ps[:].rearrange("p (h d) -> p h d", h=8),
                        )
                        vp_issued[s] = True
                        vp_step[s] = cur_g[0]

                def pv_op(h, s):
                    pvA, pvB = cur_pv[h]
                    e = ets.pop((h, s))
                    for qb in range(8):
                        grp = pvA if qb < 4 else pvB
                        # one accumulation group per psum bank: the first
                        # matmul of the head starts (zeroing the region), the
                        # last stops
                        nc.tensor.matmul(
                            grp[:, qb % 4, :],
                            e[:, qb * 128 : (qb + 1) * 128],
                            v_sb[s][:, h, :],
                            start=(s == 0 and qb % 4 == 0),
                            stop=(s == ST - 1 and qb % 4 == 3),
                        )

                def transp(t, half, xx):
                    # four query-block transposes packed into one psum tile,
                    # copied out with a single wide op
                    pt = po_p.tile([128, G], F32, tag="po", name=f"tp{t}_{half}")
                    for i in range(4):
                        qb = 4 * half + i
                        nc.tensor.matmul(
                            pt[:].bitcast(F32R)[:, i * 128 : (i + 1) * 128],
                            xx[:, qb, :],
                            ident[:],
                            is_transpose=True,
                            start=(i == 0),
                            stop=(i == 3),
                        )
                    if t == NT - 1:
                        # tail: ACT is idle once the exps are done
                        nc.scalar.activation(
                            out=xt[t][:, half * 512 : (half + 1) * 512],
                            in_=pt[:], func=AF.Copy,
                        )
                    else:
                        nc.vector.tensor_copy(
                            out=xt[t][:, half * 512 : (half + 1) * 512], in_=pt[:]
                        )

                def head_end(h):
                    t, j = divmod(h, 2)
                    pvA, pvB = cur_pv.pop(h)
                    den = den_p.tile([128, 8], F32, tag="den", name=f"den{h}")
                    nc.vector.tensor_copy(out=den[:, 0:4], in_=pvA[:, :, HD])
                    nc.vector.tensor_copy(out=den[:, 4:8], in_=pvB[:, :, HD])
                    rden = den_p.tile([128, 8], F32, tag="rden", name=f"rden{h}")
                    nc.vector.reciprocal(out=rden[:], in_=den[:])
                    if j == 0:
                        cur_x[0] = x_p.tile(
                            [128, 8, 128], F32R, tag="xall", name=f"xall{t}"
                        )
                    for qb in range(8):
                        grp = pvA if qb < 4 else pvB
                        nc.vector.tensor_scalar_mul(
                            out=cur_x[0][:, qb, j * 64 : (j + 1) * 64],
                            in0=grp[:, qb % 4, 0:HD],
                            scalar1=rden[:, qb : qb + 1],
                        )
                    if j == 1:
                        for half in range(2):
                            fill_hi.append(
                                (700, 999, lambda t=t, half=half, xx=cur_x[0]: transp(t, half, xx))
                            )

                # k tiles 1..3 as fillers (phase-2 po pool), split into three
                # light thunks so no single step swallows a full wave.
                kf_state = {}

                def kf_a(t, qc):
                    def run():
                        ps = po_p.tile([128, G], F32, tag="po", name=f"kp{t}{qc}")
                        kf_state[(t, qc)] = ps
                        k_wave_half(t, qc, ps, 0)

                    return run

                def kf_b(t, qc):
                    def run():
                        ps = kf_state.pop((t, qc))
                        k_wave_half(t, qc, ps, 1)
                        sq = k_wave_finish(t, qc, ps)
                        kf_state[(t, qc, "sq")] = sq

                    return run

                def kf_c(t, qc):
                    def run():
                        sq = kf_state.pop((t, qc, "sq"))
                        k_chain(t, qc, sq, po_p, "po")
                        k_rskT(t, qc, po_p, "po")

                    return run

                # Filler queue with rough PE-cost credits (ns).  Order
                # respects deadlines: k-t1 by step 16, k-t2 by 32, k-t3 by 48;
                # v-proj gates only the (elastic) pv drain via vp_issued.
                def k_due(t, qc):
                    # consumed by sc(2t, s) at step 16t (+4 for the qc1 half)
                    return 16 * t + 4 * qc - 8

                for qc in range(QC):
                    fill_hi.append((900, k_due(2, qc), kf_a(2, qc)))
                    fill_hi.append((900, k_due(2, qc), kf_b(2, qc)))
                    fill_hi.append((500, k_due(2, qc), kf_c(2, qc)))
                for s in range(ST):
                    for qtr in range(4):
                        fill_hi.append(
                            (500, 21 + s + qtr, lambda s=s, qtr=qtr: vp_q(s, qtr))
                        )
                for qc in range(QC):
                    fill_hi.append((900, k_due(3, qc), kf_a(3, qc)))
                    fill_hi.append((900, k_due(3, qc), kf_b(3, qc)))
                    fill_hi.append((500, k_due(3, qc), kf_c(3, qc)))

                pv_ptr = 0

                def pv_ready(ptr, g):
                    h_, s_ = divmod(ptr, 8)
                    if h_ == 0:
                        # wait ~2 steps past the v-projection wave so the pv
                        # matmul never parks at the PE queue head
                        return vp_issued[s_] and g >= vp_step[s_] + 2
                    lag = 4 if s_ == 0 else 2
                    return g >= 8 * h_ + s_ + lag

                def drain_pv(g, budget=2):
                    nonlocal pv_ptr
                    while budget > 0 and pv_ptr < 64 and pv_ready(pv_ptr, g):
                        h_, s_ = divmod(pv_ptr, 8)
                        if s_ == 0:
                            cur_pv[h_] = (
                                pv_p.tile([128, 4, HD + 1], F32, tag="pvA", name=f"pvA{h_}"),
                                pv_p.tile([128, 4, HD + 1], F32, tag="pvB", name=f"pvB{h_}"),
                            )
                        pv_op(h_, s_)
                        pv_ptr += 1
                        if s_ == ST - 1:
                            head_end(h_)
                        budget -= 1

                credit = 0.0
                RATION = 320.0  # ns of filler work per step
                for g in range(64):
                    h, s = divmod(g, 8)
                    cur_g[0] = g
                    credit += RATION
                    while fill_hi and (credit >= fill_hi[0][0] or fill_hi[0][1] <= g):
                        cost, _due, thunk = fill_hi.popleft()
                        credit = max(credit - cost, 0.0)
                        thunk()
                    sc(h, s)
                    backlog = 8 * h + s - pv_ptr
                    drain_pv(g, budget=2)

                # drain: remaining pv ops, last pair's transposes, out-proj.
                while pv_ptr < 64:
                    drain_pv(99, budget=8)
                while fill_hi:
                    fill_hi.popleft()[2]()
                # out-projection: 16 passes pipelined over 4 psum slots
                # (two halves of a psT tile + two po tiles) so the PE never
                # waits on the copy-out of the previous pass.
                for s_ in range(ST):
                    if s_ % 2 == 0:
                        stile = psT_p.tile([128, N], F32, tag="sT", name=f"ob{s_}")
                    ob = oB_p.tile([128, 2, G], F16, tag="oB", name=f"oB{s_}")
                    for coc in range(2):
                        if s_ % 2 == 0:
                            ps = stile[:, coc * 512 : (coc + 1) * 512]
                        else:
                            pot = po_p.tile([128, G], F32, tag="po", name=f"obp{s_}_{coc}")
                            ps = pot[:]
                        for t_ in range(NT):
                            nc.tensor.matmul(
                                ps,
                                xt[t_][:, s_ * 128 : (s_ + 1) * 128],
                                wo_sb[:, t_, coc * 512 : (coc + 1) * 512],
                                start=(t_ == 0),
                                stop=(t_ == NT - 1),
                            )
                        eng = nc.vector if coc == 0 else nc.scalar
                        if coc == 0:
                            nc.vector.tensor_copy(out=ob[:, 0, :], in_=ps)
                        else:
                            nc.scalar.activation(
                                out=ob[:, 1, :], in_=ps, func=AF.Copy
                            )
                    for coc in range(2):
                        nc.sync.dma_start(
                            out=out_d[:][
                                s_ * 128 : (s_ + 1) * 128, coc * 512 : (coc + 1) * 512
                            ],
                            in_=ob[:, coc, :],
                        )

    nc.compile()
    return nc


def kernel(
    query, key, value, in_proj_w, in_proj_b, logit_scale, out_w, out_b, **kw
):
    global _CACHED_NC, _LAST_IN_MAPS
    query = np.asarray(query, dtype=np.float32)
    key = np.asarray(key, dtype=np.float32)
    value = np.asarray(value, dtype=np.float32)
    in_proj_w = np.asarray(in_proj_w, dtype=np.float32)
    in_proj_b = np.asarray(in_proj_b, dtype=np.float32)
    logit_scale = np.asarray(logit_scale, dtype=np.float32)
    out_w = np.asarray(out_w, dtype=np.float32)
    out_b = np.asarray(out_b, dtype=np.float32)

    ls = np.exp(np.minimum(logit_scale.reshape(H), LOGIT_SCALE_MAX))  # (16,)
    vbias_nonzero = bool(np.any(in_proj_b[2 * C :]))

    sel8 = np.zeros((NT, 128, 8), dtype=np.float16)
    sel8T = np.zeros((8, NT, 128), dtype=np.float32)
    for t in range(NT):
        for p in range(128):
            h = 2 * t + p // 64
            sel8[t, p, h] = 1.0
            sel8T[h, t, p] = 1.0
    sel2 = np.zeros((128, 2), dtype=np.float16)
    sel2[0:64, 0] = 1.0
    sel2[64:128, 1] = 1.0
    ident = np.eye(128, dtype=np.float32)

    in_maps = []
    for c in range(8):
        b, g = c // 2, c % 2
        dims = slice(g * G, (g + 1) * G)
        ls_c = ls[g * 8 : (g + 1) * 8]  # (8,)
        in_maps.append(
            {
                "qt": np.ascontiguousarray(query[:, b, :].T, dtype=np.float16),
                "kt": np.ascontiguousarray(key[:, b, :].T, dtype=np.float16),
                "vt": np.ascontiguousarray(value[:, b, :].T, dtype=np.float16),
                "wq": np.ascontiguousarray(in_proj_w[0 * C :, :][dims, :].T, dtype=np.float16),
                "wk": np.ascontiguousarray(in_proj_w[1 * C :, :][dims, :].T, dtype=np.float16),
                "wv": np.ascontiguousarray(in_proj_w[2 * C :, :][dims, :].T, dtype=np.float16),
                "wo": np.ascontiguousarray(out_w[:, dims].T, dtype=np.float16),
                "bq": np.ascontiguousarray(
                    in_proj_b[0 * C :][dims].reshape(NT, 128).T, dtype=np.float32
                ),
                "bk": np.ascontiguousarray(
                    in_proj_b[1 * C :][dims].reshape(NT, 128).T, dtype=np.float32
                ),
                "bv": np.ascontiguousarray(
                    in_proj_b[2 * C :][dims].reshape(1, G), dtype=np.float16
                ),
                "sel8": sel8,
                "sel2": sel2,
                "sel8T": sel8T,
                "lnls": np.log(ls_c).reshape(NT, 2).T.astype(np.float32).copy(),
                "lsbias": np.repeat(-ls_c.reshape(1, 8), 128, axis=0).astype(np.float32),
                "ident": ident,
                "ones1": np.ones((1, 128), dtype=np.float16),
            }
        )

    _LAST_IN_MAPS = in_maps
    if vbias_nonzero not in _CACHED_NC:
        _CACHED_NC[vbias_nonzero] = build_nc(vbias_nonzero)
    res = run_bass_kernel_spmd(
        _CACHED_NC[vbias_nonzero], in_maps, core_ids=list(range(8))
    )

    out = np.zeros((N, B, C), dtype=np.float32)
    for c in range(8):
        b = c // 2
        out[:, b, :] += res.results[c]["out"].astype(np.float32)
    out += out_b.reshape(1, 1, C)
    return out


# revision 31
# speedup vs baseline: 1.0012x; 1.0012x over previous
# Trainium2 Bass kernel for nn_CustomAttention (cosine-sim multi-head attention).
#
# Sharding over 8 cores: core c handles batch b = c//2 and head group
# g = c%2 (8 of 16 heads, 512 feature dims).  Each core computes its heads'
# q/k/v projections (Megatron column-parallel), cosine-sim attention, and a
# partial output projection (row-parallel over its 512 dims).  The host sums
# the two partial outputs per batch and adds out_b.
#
# All heavy matmuls run in fp16 (1 cycle/row on the PE at any output width),
# with fp32 PSUM accumulation.  Layout highlights:
#   qT/kT: (dims=512, seq=1024) as 4 tiles of (128, 1024); head dims on
#          partitions so the scores matmul contracts head_dim on partitions.
#   scores are computed transposed, sT[k, q]; the per-key ls/||k|| factor and
#   the -ls bias fold into the exp() activation as per-partition APs.
#   1/||q|| and ls/||k|| come from exp(-0.5*ln(ssq) [+ ln ls]) so the whole
#   kernel uses a single activation table (ln+exp) -- no table reloads.
#   p@v runs transposed: x[q, d] = eT[k, q-block]^T @ v[k, d|1]; the appended
#   ones-column of v gives the softmax denominator, applied as a per-partition
#   (per-query) scalar during the PSUM->SBUF copy.  x tiles are PE-transposed
#   back to (dims, seq) for the out-proj.
#   out-proj is split into two half-contractions: t0+t1 overlaps the attention
#   stream, t2+t3 runs at the tail (the A half folded into the PSUM
#   accumulation via an identity matmul; copy-out split between DVE and ACT).
#
# Scheduling: engine queues drain strictly in issue order, so issue order ==
# execution order.  The attention main loop is ACT-paced (64 wide exps); the
# PE stream weaves scores, p@v, the v projection, the k projection for tiles
# t1..t3 (only q and k-t0 are done up front), x transposes and the first
# out-proj half into the gaps as "fillers".  GPSIMD cannot touch PSUM, so all
# PSUM->SBUF traffic is on DVE (and ACT Copy in the tail).

import math
import sys
from collections import deque

import numpy as np

sys.path.insert(0, "/opt/trn_rl_repo")

import concourse.bass as bass
import concourse.tile as tile
from concourse import bacc, mybir
from concourse.bass_utils import run_bass_kernel_spmd
from concourse.hw_specs import get_activation_tables

N = 1024  # sequence length
B = 4  # batch
C = 1024  # channels
H = 16  # total heads
HD = 64  # head dim
G = 512  # dims per core (8 heads)
NT = 4  # (128, N) tiles of qT/kT per core
CC = 8  # contraction chunks of 128 over C
ST = 8  # seq tiles of 128
QC = 2  # seq chunks of 512
LOGIT_SCALE_MAX = math.log(1.0 / 0.01)

F32 = mybir.dt.float32
F32R = mybir.dt.float32r
F16 = mybir.dt.float16
AF = mybir.ActivationFunctionType

_CACHED_NC = {}
_LAST_IN_MAPS = None


def build_nc(vbias_nonzero):
    nc = bacc.Bacc("TRN2", target_bir_lowering=False)

    qt_d = nc.declare_dram_parameter("qt", [C, N], F16, isOutput=False)
    kt_d = nc.declare_dram_parameter("kt", [C, N], F16, isOutput=False)
    vt_d = nc.declare_dram_parameter("vt", [C, N], F16, isOutput=False)
    wq_d = nc.declare_dram_parameter("wq", [C, G], F16, isOutput=False)
    wk_d = nc.declare_dram_parameter("wk", [C, G], F16, isOutput=False)
    wv_d = nc.declare_dram_parameter("wv", [C, G], F16, isOutput=False)
    wo_d = nc.declare_dram_parameter("wo", [G, C], F16, isOutput=False)
    bq_d = nc.declare_dram_parameter("bq", [128, NT], F32, isOutput=False)
    bk_d = nc.declare_dram_parameter("bk", [128, NT], F32, isOutput=False)
    bv_d = nc.declare_dram_parameter("bv", [1, G], F16, isOutput=False)
    sel8_d = nc.declare_dram_parameter("sel8", [NT, 128, 8], F16, isOutput=False)
    sel2_d = nc.declare_dram_parameter("sel2", [128, 2], F16, isOutput=False)
    sel8T_d = nc.declare_dram_parameter("sel8T", [8, NT, 128], F32R, isOutput=False)
    lnls_d = nc.declare_dram_parameter("lnls", [2, NT], F32, isOutput=False)
    lsbias_d = nc.declare_dram_parameter("lsbias", [128, 8], F32, isOutput=False)
    ident_d = nc.declare_dram_parameter("ident", [128, 128], F32R, isOutput=False)
    ones1_d = nc.declare_dram_parameter("ones1", [1, 128], F16, isOutput=False)
    out_d = nc.declare_dram_parameter("out", [N, C], F16, isOutput=True)

    qt_r = qt_d[:].rearrange("(cc p) n -> cc p n", p=128)
    kt_r = kt_d[:].rearrange("(cc p) n -> cc p n", p=128)
    vt_r = vt_d[:].rearrange("(cc p) n -> cc p n", p=128)
    wq_r = wq_d[:].rearrange("(g cc p) o -> g p cc o", g=2, p=128)
    wk_r = wk_d[:].rearrange("(g cc p) o -> g p cc o", g=2, p=128)
    wv_r = wv_d[:].rearrange("(g cc p) o -> g p cc o", g=2, p=128)

    # pre-load the ln+exp activation table once; every ACT op in this kernel
    # (Ln, Exp, Copy) is servable from it, so the auto-inserted loads (which
    # thrash between exp-only and ln-only tables) are avoided.
    table_names = list(get_activation_tables(nc.m.arch).keys())
    lnexp_id = table_names.index("natural_log_exp_and_others")

    with tile.TileContext(nc) as tc:
        nc.scalar.add_instruction(
            mybir.InstLoadActFuncSet(
                name=nc.get_next_instruction_name(), ins=[], outs=[],
                act_func_set_id=lnexp_id,
            )
        )
        with (
            tc.tile_pool(name="consts", bufs=1) as consts,
            tc.tile_pool(name="wo_p", bufs=1) as wo_p,
            tc.tile_pool(name="w_p", bufs=3) as w_p,
            tc.tile_pool(name="acts", bufs=16) as acts,
            tc.tile_pool(name="big", bufs=1) as big,
            tc.tile_pool(name="sq_p", bufs=2) as sq_p,
            tc.tile_pool(name="stats", bufs=1) as stats,
            tc.tile_pool(name="lssq_p", bufs=2) as lssq_p,
            tc.tile_pool(name="eT_p", bufs=26) as eT_p,
            tc.tile_pool(name="x_p", bufs=3) as x_p,
            tc.tile_pool(name="den_p", bufs=2) as den_p,
            tc.tile_pool(name="oB_p", bufs=4) as oB_p,
        ):
            # ---- persistent tiles ----
            qT = [big.tile([128, N], F16, tag=f"qT{t}", name=f"qT{t}") for t in range(NT)]
            kT = [big.tile([128, N], F16, tag=f"kT{t}", name=f"kT{t}") for t in range(NT)]
            v_sb = [big.tile([128, 8, HD + 1], F16, tag=f"v{s}", name=f"v{s}") for s in range(ST)]
            xt = [big.tile([128, N], F16, tag=f"xt{t}", name=f"xt{t}") for t in range(NT)]
            rskT = stats.tile([128, ST, 8], F32)
            rsq = stats.tile([8, N], F32R)
            rsk_t = [
                stats.tile([2, N], F32R, tag=f"rsk{t}", name=f"rsk{t}")
                for t in range(NT)
            ]
            # ones column of v (softmax denominator); SBUF-only so GPSIMD ok.
            for s in range(ST):
                nc.gpsimd.memset(v_sb[s][:, :, HD], 1.0)

            # ---- DMA stream (single SP queue; issue order = transfer order) ----
            # q stream first: the first projection wave starts ~2us in.
            wq_sb = w_p.tile([128, CC, G], F16, tag="w", name="wq")
            qch = []
            a0 = acts.tile([128, N], F16, tag="act", name="qt0")
            nc.sync.dma_start(out=a0[:], in_=qt_r[0])
            qch.append(a0)
            nc.sync.dma_start(out=wq_sb[:, 0:4, :], in_=wq_r[0])
            for cc in range(1, 4):
                a = acts.tile([128, N], F16, tag="act", name=f"qt{cc}")
                nc.sync.dma_start(out=a[:], in_=qt_r[cc])
                qch.append(a)
            nc.sync.dma_start(out=wq_sb[:, 4:8, :], in_=wq_r[1])
            for cc in range(4, 8):
                a = acts.tile([128, N], F16, tag="act", name=f"qt{cc}")
                nc.sync.dma_start(out=a[:], in_=qt_r[cc])
                qch.append(a)

            # consts needed during the q projection
            sel8 = consts.tile([128, NT, 8], F16)
            nc.sync.dma_start(out=sel8[:], in_=sel8_d[:].rearrange("t p e -> p t e"))
            sel2 = consts.tile([128, 2], F16)
            nc.sync.dma_start(out=sel2[:], in_=sel2_d[:])
            bq_sb = consts.tile([128, NT], F32)
            nc.sync.dma_start(out=bq_sb[:], in_=bq_d[:])

            wk_sb = w_p.tile([128, CC, G], F16, tag="w", name="wk")
            nc.sync.dma_start(out=wk_sb[:, 0:4, :], in_=wk_r[0])
            nc.sync.dma_start(out=wk_sb[:, 4:8, :], in_=wk_r[1])
            kch = []
            for cc in range(CC):
                a = acts.tile([128, N], F16, tag="act", name=f"kt{cc}")
                nc.sync.dma_start(out=a[:], in_=kt_r[cc])
                kch.append(a)

            sel8T = consts.tile([8, NT, 128], F32R)
            nc.sync.dma_start(out=sel8T[:], in_=sel8T_d[:])
            bk_sb = consts.tile([128, NT], F32)
            nc.sync.dma_start(out=bk_sb[:], in_=bk_d[:])
            lnls = consts.tile([2, NT], F32)
            nc.sync.dma_start(out=lnls[:], in_=lnls_d[:])
            ident = consts.tile([128, 128], F32R)
            nc.sync.dma_start(out=ident[:], in_=ident_d[:])
            lsbias = consts.tile([128, 8], F32)
            nc.sync.dma_start(out=lsbias[:], in_=lsbias_d[:])

            wv_sb = w_p.tile([128, CC, G], F16, tag="w", name="wv")
            nc.sync.dma_start(out=wv_sb[:, 0:4, :], in_=wv_r[0])
            nc.sync.dma_start(out=wv_sb[:, 4:8, :], in_=wv_r[1])
            vch = []
            for cc in range(CC):
                a = acts.tile([128, N], F16, tag="act", name=f"vt{cc}")
                nc.sync.dma_start(out=a[:], in_=vt_r[cc])
                vch.append(a)

            ones1 = consts.tile([1, 128], F16)
            nc.sync.dma_start(out=ones1[:], in_=ones1_d[:])
            bv_sb = consts.tile([1, G], F16)
            nc.sync.dma_start(out=bv_sb[:], in_=bv_d[:])
            wo_sb = wo_p.tile([128, NT, C], F16)
            nc.sync.dma_start(
                out=wo_sb[:], in_=wo_d[:].rearrange("(t p) c -> p t c", p=128)
            )

            # ---- phase 1: q projection (full) + k projection tile 0 ----
            deferred = deque()

            def flush(n=99):
                for _ in range(min(n, len(deferred))):
                    deferred.popleft()()

            with (
                tc.tile_pool(name="pp", bufs=4, space="PSUM") as pp,
                tc.tile_pool(name="pssq", bufs=2, space="PSUM") as pssq_p,
                tc.tile_pool(name="pbc", bufs=2, space="PSUM") as pbc,
            ):
                # q projection in 4 pair-tile waves; global per-head ssq.
                def q_wave(qc, tp, ssq_half):
                    pst = [
                        pp.tile([128, G], F32, tag="proj", name=f"qp{qc}{tp}{i}")
                        for i in range(2)
                    ]
                    for cc in range(CC):
                        for ti in range(2):
                            t = 2 * tp + ti
                            nc.tensor.matmul(
                                pst[ti][:],
                                wq_sb[:, cc, t * 128 : (t + 1) * 128],
                                qch[cc][:, qc * 512 : (qc + 1) * 512],
                                start=(cc == 0),
                                stop=(cc == CC - 1),
                            )
                    for ti in range(2):
                        t = 2 * tp + ti
                        nc.vector.tensor_scalar_add(
                            out=qT[t][:, qc * 512 : (qc + 1) * 512],
                            in0=pst[ti][:],
                            scalar1=bq_sb[:, t : t + 1],
                        )
                        sq = sq_p.tile([128, G], F16, tag="sq", name=f"sq{t}{qc}")
                        nc.vector.tensor_mul(
                            out=sq[:],
                            in0=qT[t][:, qc * 512 : (qc + 1) * 512],
                            in1=qT[t][:, qc * 512 : (qc + 1) * 512],
                        )

                        def ssq_mm(t=t, sq=sq):
                            nc.tensor.matmul(
                                ssq_half[:],
                                sel8[:, t, :],
                                sq[:],
                                start=(t == 0),
                                stop=(t == NT - 1),
                            )

                        deferred.append(ssq_mm)

                def q_norm_thunk(ssq_half, qc):
                    def run():
                        lssq = lssq_p.tile([8, G], F32, tag="lssq", name=f"lsq{qc}")
                        nc.scalar.activation(out=lssq[:], in_=ssq_half[:], func=AF.Ln)
                        nc.scalar.activation(
                            out=rsq[:, qc * 512 : (qc + 1) * 512],
                            in_=lssq[:], func=AF.Exp, scale=-0.5,
                        )

                    return run

                def bc_thunk(t, qc):
                    def run():
                        pb = pbc.tile([128, G], F32, tag="bc", name=f"bc{t}{qc}")
                        nc.tensor.matmul(
                            pb[:],
                            sel8T[:, t, :],
                            rsq[:, qc * 512 : (qc + 1) * 512],
                            start=True,
                            stop=True,
                        )
                        nc.vector.tensor_mul(
                            out=qT[t][:, qc * 512 : (qc + 1) * 512],
                            in0=qT[t][:, qc * 512 : (qc + 1) * 512],
                            in1=pb[:],
                        )

                    return run

                for qc in range(QC):
                    ssq_half = pssq_p.tile([8, G], F32, tag="ssq", name=f"sq_q{qc}")
                    for tp in range(2):
                        q_wave(qc, tp, ssq_half)
                        flush(2)
                    flush()
                    deferred.append(q_norm_thunk(ssq_half, qc))
                    for t in range(NT):
                        deferred.append(bc_thunk(t, qc))

                # k projection: single-tile waves.  Tile 0 runs here (prefix);
                # tiles 1..3 run as fillers inside the attention loop using the
                # phase-2 "po" pool.  Per-(tile, half) ssq + norm chains.
                def k_wave_half(t, qc, ps, half):
                    for cc in range(4 * half, 4 * half + 4):
                        nc.tensor.matmul(
                            ps[:],
                            wk_sb[:, cc, t * 128 : (t + 1) * 128],
                            kch[cc][:, qc * 512 : (qc + 1) * 512],
                            start=(cc == 0),
                            stop=(cc == CC - 1),
                        )

                def k_wave_finish(t, qc, ps):
                    nc.vector.tensor_scalar_add(
                        out=kT[t][:, qc * 512 : (qc + 1) * 512],
                        in0=ps[:],
                        scalar1=bk_sb[:, t : t + 1],
                    )
                    sq = sq_p.tile([128, G], F16, tag="sq", name=f"ksq{t}{qc}")
                    nc.vector.tensor_mul(
                        out=sq[:],
                        in0=kT[t][:, qc * 512 : (qc + 1) * 512],
                        in1=kT[t][:, qc * 512 : (qc + 1) * 512],
                    )
                    return sq

                def k_wave(t, qc, pool, ptag):
                    ps = pool.tile([128, G], F32, tag=ptag, name=f"kp{t}{qc}")
                    k_wave_half(t, qc, ps, 0)
                    k_wave_half(t, qc, ps, 1)
                    return k_wave_finish(t, qc, ps)

                def k_chain(t, qc, sq, pool, ptag):
                    # ssq (2 rows) -> sbuf bounce -> ln -> exp into rsk_t[t].
                    # The DVE bounce frees the PSUM tile immediately instead of
                    # holding it until ACT's (busy, in-order) queue reaches Ln.
                    ps = pool.tile([128, G], F32, tag=ptag, name=f"kn{t}{qc}")
                    nc.tensor.matmul(
                        ps[0:2, :], sel2[:], sq[:], start=True, stop=True
                    )
                    ssb = lssq_p.tile([2, G], F32, tag="ssb", name=f"skb{t}{qc}")
                    nc.vector.tensor_copy(out=ssb[:], in_=ps[0:2, :])
                    lssq = lssq_p.tile([2, G], F32, tag="lssq", name=f"lsk{t}{qc}")
                    nc.scalar.activation(out=lssq[:], in_=ssb[:], func=AF.Ln)
                    nc.scalar.activation(
                        out=rsk_t[t][:, qc * 512 : (qc + 1) * 512],
                        in_=lssq[:], func=AF.Exp, scale=-0.5,
                        bias=lnls[:, t : t + 1],
                    )

                def k_rskT(t, qc, pool, ptag):
                    # transpose rsk[2t:2t+2, half] into rskT[:, s, 2t:2t+2]
                    # for the 4 key tiles s of this half, all in one psum tile.
                    pt = pool.tile([128, G], F32, tag=ptag, name=f"krt{t}{qc}")
                    for i in range(4):
                        s = 4 * qc + i
                        nc.tensor.matmul(
                            pt[:].bitcast(F32R)[:, 2 * i : 2 * i + 2],
                            rsk_t[t][:, s * 128 : (s + 1) * 128],
                            ident[0:2, 0:2],
                            is_transpose=True,
                            start=(i == 0),
                            stop=(i == 3),
                        )
                    nc.vector.tensor_copy(
                        out=rskT[:, 4 * qc : 4 * qc + 4, 2 * t : 2 * t + 2],
                        in_=pt[:, 0:8].rearrange("p (s e) -> p s e", e=2),
                    )

                for t in range(2):
                    for qc in range(QC):
                        sq = k_wave(t, qc, pp, "proj")
                        flush(2)
                        k_chain(t, qc, sq, pbc, "bc")
                        k_rskT(t, qc, pbc, "bc")
                flush()

            # ---- phase 2: attention + v proj + k t1..3 + out-projection ----
            with (
                tc.tile_pool(name="psT", bufs=2, space="PSUM") as psT_p,
                tc.tile_pool(name="pv", bufs=1, space="PSUM") as pv_p,
                tc.tile_pool(name="po", bufs=2, space="PSUM") as po_p,
            ):
                ets = {}
                cur_pv = {}
                cur_x = [None] * 8
                vp_issued = [False] * ST
                vp_step = [99] * ST
                cur_g = [0]
                fill_hi = deque()
                fill_lo = deque()

                def sc(h, s):
                    t, j = divmod(h, 2)
                    st = psT_p.tile([128, N], F32, tag="sT", name=f"sT{h}_{s}")
                    for qc in range(QC):
                        nc.tensor.matmul(
                            st[:, qc * 512 : (qc + 1) * 512],
                            kT[t][j * 64 : (j + 1) * 64, s * 128 : (s + 1) * 128],
                            qT[t][j * 64 : (j + 1) * 64, qc * 512 : (qc + 1) * 512],
                            start=True,
                            stop=True,
                        )
                    e = eT_p.tile([128, N], F16, tag="eT", name=f"eT{h}_{s}")
                    nc.scalar.activation(
                        out=e[:], in_=st[:], func=AF.Exp,
                        bias=lsbias[:, h : h + 1],
                        scale=rskT[:, s, h : h + 1],
                    )
                    ets[(h, s)] = e

                vp_state = {}

                def vp_half(s, half):
                    if half == 0:
                        ps = po_p.tile([128, G], F32, tag="po", name=f"vp{s}")
                        vp_state[s] = ps
                    else:
                        ps = vp_state.pop(s)
                    for cc in range(4 * half, 4 * half + 4):
                        nc.tensor.matmul(
                            ps[:],
                            vch[cc][:, s * 128 : (s + 1) * 128],
                            wv_sb[:, cc, :],
                            start=(cc == 0),
                            stop=(not vbias_nonzero and cc == CC - 1),
                        )
                    if half == 1:
                        if vbias_nonzero:
                            nc.tensor.matmul(
                                ps[:], ones1[:], bv_sb[:], start=False, stop=True
                            )
                        nc.vector.tensor_copy(
                            out=v_sb[s][:, :, 0:HD],
                            in_=ps[:].rearrange("p (h d) -> p h d", h=8),
                        )
                        vp_issued[s] = True
                        vp_step[s] = cur_g[0]

                def pv_op(h, s):
                    pvA, pvB = cur_pv[h]
                    e = ets.pop((h, s))
                    for qb in range(8):
                        grp = pvA if qb < 4 else pvB
                        # one accumulation group per psum bank: the first
                        # matmul of the head starts (zeroing the region), the
                        # last stops
                        nc.tensor.matmul(
                            grp[:, qb % 4, :],
                            e[:, qb * 128 : (qb + 1) * 128],
                            v_sb[s][:, h, :],
                            start=(s == 0 and qb % 4 == 0),
                            stop=(s == ST - 1 and qb % 4 == 3),
                        )

                def transp(t, half, xx):
                    # four query-block transposes packed into one psum tile,
                    # copied out with a single wide op
                    pt = po_p.tile([128, G], F32, tag="po", name=f"tp{t}_{half}")
                    for i in range(4):
                        qb = 4 * half + i
                        nc.tensor.matmul(
                            pt[:].bitcast(F32R)[:, i * 128 : (i + 1) * 128],
                            xx[:, qb, :],
                            ident[:],
                            is_transpose=True,
                            start=(i == 0),
                            stop=(i == 3),
                        )
                    if t == NT - 1:
                        # tail: ACT is idle once the exps are done
                        nc.scalar.activation(
                            out=xt[t][:, half * 512 : (half + 1) * 512],
                            in_=pt[:], func=AF.Copy,
                        )
                    else:
                        nc.vector.tensor_copy(
                            out=xt[t][:, half * 512 : (half + 1) * 512], in_=pt[:]
                        )

                def head_end(h):
                    t, j = divmod(h, 2)
                    pvA, pvB = cur_pv.pop(h)
                    den = den_p.tile([128, 8], F32, tag="den", name=f"den{h}")
                    nc.vector.tensor_copy(out=den[:, 0:4], in_=pvA[:, :, HD])
                    nc.vector.tensor_copy(out=den[:, 4:8], in_=pvB[:, :, HD])
                    rden = den_p.tile([128, 8], F32, tag="rden", name=f"rden{h}")
                    nc.vector.reciprocal(out=rden[:], in_=den[:])
                    if j == 0:
                        cur_x[0] = x_p.tile(
                            [128, 8, 128], F32R, tag="xall", name=f"xall{t}"
                        )
                    for qb in range(8):
                        grp = pvA if qb < 4 else pvB
                        nc.vector.tensor_scalar_mul(
                            out=cur_x[0][:, qb, j * 64 : (j + 1) * 64],
                            in0=grp[:, qb % 4, 0:HD],
                            scalar1=rden[:, qb : qb + 1],
                        )
                    if j == 1:
                        for half in range(2):
                            fill_hi.append(
                                (700, 999, lambda t=t, half=half, xx=cur_x[0]: transp(t, half, xx))
                            )

                # k tiles 1..3 as fillers (phase-2 po pool), split into three
                # light thunks so no single step swallows a full wave.
                kf_state = {}

                def kf_a(t, qc):
                    def run():
                        ps = po_p.tile([128, G], F32, tag="po", name=f"kp{t}{qc}")
                        kf_state[(t, qc)] = ps
                        k_wave_half(t, qc, ps, 0)

                    return run

                def kf_b(t, qc):
                    def run():
                        ps = kf_state.pop((t, qc))
                        k_wave_half(t, qc, ps, 1)
                        sq = k_wave_finish(t, qc, ps)
                        kf_state[(t, qc, "sq")] = sq

                    return run

                def kf_c(t, qc):
                    def run():
                        sq = kf_state.pop((t, qc, "sq"))
                        k_chain(t, qc, sq, po_p, "po")
                        k_rskT(t, qc, po_p, "po")

                    return run

                # Filler queue with rough PE-cost credits (ns).  Order
                # respects deadlines: k-t1 by step 16, k-t2 by 32, k-t3 by 48;
                # v-proj gates only the (elastic) pv drain via vp_issued.
                def k_due(t, qc):
                    # consumed by sc(2t, s) at step 16t (+4 for the qc1 half)
                    return 16 * t + 4 * qc - 8

                for qc in range(QC):
                    fill_hi.append((900, k_due(2, qc), kf_a(2, qc)))
                    fill_hi.append((900, k_due(2, qc), kf_b(2, qc)))
                    fill_hi.append((500, k_due(2, qc), kf_c(2, qc)))
                for s in range(ST):
                    fill_hi.append((1000, 23 + s, lambda s=s: vp_half(s, 0)))
                    fill_hi.append((1000, 24 + s, lambda s=s: vp_half(s, 1)))
                for qc in range(QC):
                    fill_hi.append((900, k_due(3, qc), kf_a(3, qc)))
                    fill_hi.append((900, k_due(3, qc), kf_b(3, qc)))
                    fill_hi.append((500, k_due(3, qc), kf_c(3, qc)))

                pv_ptr = 0

                def pv_ready(ptr, g):
                    h_, s_ = divmod(ptr, 8)
                    if h_ == 0:
                        # wait ~2 steps past the v-projection wave so the pv
                        # matmul never parks at the PE queue head
                        return vp_issued[s_] and g >= vp_step[s_] + 2
                    lag = 4 if s_ == 0 else 2
                    return g >= 8 * h_ + s_ + lag

                def drain_pv(g, budget=2):
                    nonlocal pv_ptr
                    while budget > 0 and pv_ptr < 64 and pv_ready(pv_ptr, g):
                        h_, s_ = divmod(pv_ptr, 8)
                        if s_ == 0:
                            cur_pv[h_] = (
                                pv_p.tile([128, 4, HD + 1], F32, tag="pvA", name=f"pvA{h_}"),
                                pv_p.tile([128, 4, HD + 1], F32, tag="pvB", name=f"pvB{h_}"),
                            )
                        pv_op(h_, s_)
                        pv_ptr += 1
                        if s_ == ST - 1:
                            head_end(h_)
                        budget -= 1

                credit = 0.0
                RATION = 320.0  # ns of filler work per step
                for g in range(64):
                    h, s = divmod(g, 8)
                    cur_g[0] = g
                    credit += RATION
                    while fill_hi and (credit >= fill_hi[0][0] or fill_hi[0][1] <= g):
                        cost, _due, thunk = fill_hi.popleft()
                        credit = max(credit - cost, 0.0)
                        thunk()
                    sc(h, s)
                    backlog = 8 * h + s - pv_ptr
                    drain_pv(g, budget=2)

                # drain: remaining pv ops, last pair's transposes, out-proj.
                while pv_ptr < 64:
                    drain_pv(99, budget=8)
                while fill_hi:
                    fill_hi.popleft()[2]()
                # out-projection: 16 passes pipelined over 4 psum slots
                # (two halves of a psT tile + two po tiles) so the PE never
                # waits on the copy-out of the previous pass.
                for s_ in range(ST):
                    if s_ % 2 == 0:
                        stile = psT_p.tile([128, N], F32, tag="sT", name=f"ob{s_}")
                    ob = oB_p.tile([128, 2, G], F16, tag="oB", name=f"oB{s_}")
                    for coc in range(2):
                        if s_ % 2 == 0:
                            ps = stile[:, coc * 512 : (coc + 1) * 512]
                        else:
                            pot = po_p.tile([128, G], F32, tag="po", name=f"obp{s_}_{coc}")
                            ps = pot[:]
                        for t_ in range(NT):
                            nc.tensor.matmul(
                                ps,
                                xt[t_][:, s_ * 128 : (s_ + 1) * 128],
                                wo_sb[:, t_, coc * 512 : (coc + 1) * 512],
                                start=(t_ == 0),
                                stop=(t_ == NT - 1),
                            )
                        eng = nc.vector if coc == 0 else nc.scalar
                        if coc == 0:
                            nc.vector.tensor_copy(out=ob[:, 0, :], in_=ps)
                        else:
                            nc.scalar.activation(
                                out=ob[:, 1, :], in_=ps, func=AF.Copy
                            )
                    for coc in range(2):
                        nc.sync.dma_start(
                            out=out_d[:][
                                s_ * 128 : (s_ + 1) * 128, coc * 512 : (coc + 1) * 512
                            ],
                            in_=ob[:, coc, :],
                        )

    nc.compile()
    return nc


def kernel(
    query, key, value, in_proj_w, in_proj_b, logit_scale, out_w, out_b, **kw
):
    global _CACHED_NC, _LAST_IN_MAPS
    query = np.asarray(query, dtype=np.float32)
    key = np.asarray(key, dtype=np.float32)
    value = np.asarray(value, dtype=np.float32)
    in_proj_w = np.asarray(in_proj_w, dtype=np.float32)
    in_proj_b = np.asarray(in_proj_b, dtype=np.float32)
    logit_scale = np.asarray(logit_scale, dtype=np.float32)
    out_w = np.asarray(out_w, dtype=np.float32)
    out_b = np.asarray(out_b, dtype=np.float32)

    ls = np.exp(np.minimum(logit_scale.reshape(H), LOGIT_SCALE_MAX))  # (16,)
    vbias_nonzero = bool(np.any(in_proj_b[2 * C :]))

    sel8 = np.zeros((NT, 128, 8), dtype=np.float16)
    sel8T = np.zeros((8, NT, 128), dtype=np.float32)
    for t in range(NT):
        for p in range(128):
            h = 2 * t + p // 64
            sel8[t, p, h] = 1.0
            sel8T[h, t, p] = 1.0
    sel2 = np.zeros((128, 2), dtype=np.float16)
    sel2[0:64, 0] = 1.0
    sel2[64:128, 1] = 1.0
    ident = np.eye(128, dtype=np.float32)

    in_maps = []
    for c in range(8):
        b, g = c // 2, c % 2
        dims = slice(g * G, (g + 1) * G)
        ls_c = ls[g * 8 : (g + 1) * 8]  # (8,)
        in_maps.append(
            {
                "qt": np.ascontiguousarray(query[:, b, :].T, dtype=np.float16),
                "kt": np.ascontiguousarray(key[:, b, :].T, dtype=np.float16),
                "vt": np.ascontiguousarray(value[:, b, :].T, dtype=np.float16),
                "wq": np.ascontiguousarray(in_proj_w[0 * C :, :][dims, :].T, dtype=np.float16),
                "wk": np.ascontiguousarray(in_proj_w[1 * C :, :][dims, :].T, dtype=np.float16),
                "wv": np.ascontiguousarray(in_proj_w[2 * C :, :][dims, :].T, dtype=np.float16),
                "wo": np.ascontiguousarray(out_w[:, dims].T, dtype=np.float16),
                "bq": np.ascontiguousarray(
                    in_proj_b[0 * C :][dims].reshape(NT, 128).T, dtype=np.float32
                ),
                "bk": np.ascontiguousarray(
                    in_proj_b[1 * C :][dims].reshape(NT, 128).T, dtype=np.float32
                ),
                "bv": np.ascontiguousarray(
                    in_proj_b[2 * C :][dims].reshape(1, G), dtype=np.float16
                ),
                "sel8": sel8,
                "sel2": sel2,
                "sel8T": sel8T,
                "lnls": np.log(ls_c).reshape(NT, 2).T.astype(np.float32).copy(),
                "lsbias": np.repeat(-ls_c.reshape(1, 8), 128, axis=0).astype(np.float32),
                "ident": ident,
                "ones1": np.ones((1, 128), dtype=np.float16),
            }
        )

    _LAST_IN_MAPS = in_maps
    if vbias_nonzero not in _CACHED_NC:
        _CACHED_NC[vbias_nonzero] = build_nc(vbias_nonzero)
    res = run_bass_kernel_spmd(
        _CACHED_NC[vbias_nonzero], in_maps, core_ids=list(range(8))
    )

    out = np.zeros((N, B, C), dtype=np.float32)
    for c in range(8):
        b = c // 2
        out[:, b, :] += res.results[c]["out"].astype(np.float32)
    out += out_b.reshape(1, 1, C)
    return out


# revision 35
# speedup vs baseline: 1.0038x; 1.0026x over previous
# Trainium2 Bass kernel for nn_CustomAttention (cosine-sim multi-head attention).
#
# Sharding over 8 cores: core c handles batch b = c//2 and head group
# g = c%2 (8 of 16 heads, 512 feature dims).  Each core computes its heads'
# q/k/v projections (Megatron column-parallel), cosine-sim attention, and a
# partial output projection (row-parallel over its 512 dims).  The host sums
# the two partial outputs per batch and adds out_b.
#
# All heavy matmuls run in fp16 (1 cycle/row on the PE at any output width),
# with fp32 PSUM accumulation.  Layout highlights:
#   qT/kT: (dims=512, seq=1024) as 4 tiles of (128, 1024); head dims on
#          partitions so the scores matmul contracts head_dim on partitions.
#   scores are computed transposed, sT[k, q]; the per-key ls/||k|| factor and
#   the -ls bias fold into the exp() activation as per-partition APs.
#   1/||q|| and ls/||k|| come from exp(-0.5*ln(ssq) [+ ln ls]) so the whole
#   kernel uses a single activation table (ln+exp) -- no table reloads.
#   p@v runs transposed: x[q, d] = eT[k, q-block]^T @ v[k, d|1]; the appended
#   ones-column of v gives the softmax denominator, applied as a per-partition
#   (per-query) scalar during the PSUM->SBUF copy.  x tiles are PE-transposed
#   back to (dims, seq) for the out-proj.
#   out-proj is split into two half-contractions: t0+t1 overlaps the attention
#   stream, t2+t3 runs at the tail (the A half folded into the PSUM
#   accumulation via an identity matmul; copy-out split between DVE and ACT).
#
# Scheduling: engine queues drain strictly in issue order, so issue order ==
# execution order.  The attention main loop is ACT-paced (64 wide exps); the
# PE stream weaves scores, p@v, the v projection, the k projection for tiles
# t1..t3 (only q and k-t0 are done up front), x transposes and the first
# out-proj half into the gaps as "fillers".  GPSIMD cannot touch PSUM, so all
# PSUM->SBUF traffic is on DVE (and ACT Copy in the tail).

import math
import sys
from collections import deque

import numpy as np

sys.path.insert(0, "/opt/trn_rl_repo")

import concourse.bass as bass
import concourse.tile as tile
from concourse import bacc, mybir
from concourse.bass_utils import run_bass_kernel_spmd
from concourse.hw_specs import get_activation_tables

N = 1024  # sequence length
B = 4  # batch
C = 1024  # channels
H = 16  # total heads
HD = 64  # head dim
G = 512  # dims per core (8 heads)
NT = 4  # (128, N) tiles of qT/kT per core
CC = 8  # contraction chunks of 128 over C
ST = 8  # seq tiles of 128
QC = 2  # seq chunks of 512
LOGIT_SCALE_MAX = math.log(1.0 / 0.01)

F32 = mybir.dt.float32
F32R = mybir.dt.float32r
F16 = mybir.dt.float16
AF = mybir.ActivationFunctionType

_CACHED_NC = {}
_LAST_IN_MAPS = None


def build_nc(vbias_nonzero):
    nc = bacc.Bacc("TRN2", target_bir_lowering=False)

    qt_d = nc.declare_dram_parameter("qt", [C, N], F16, isOutput=False)
    kt_d = nc.declare_dram_parameter("kt", [C, N], F16, isOutput=False)
    vt_d = nc.declare_dram_parameter("vt", [C, N], F16, isOutput=False)
    wq_d = nc.declare_dram_parameter("wq", [C, G], F16, isOutput=False)
    wk_d = nc.declare_dram_parameter("wk", [C, G], F16, isOutput=False)
    wv_d = nc.declare_dram_parameter("wv", [C, G], F16, isOutput=False)
    wo_d = nc.declare_dram_parameter("wo", [G, C], F16, isOutput=False)
    bq_d = nc.declare_dram_parameter("bq", [128, NT], F32, isOutput=False)
    bk_d = nc.declare_dram_parameter("bk", [128, NT], F32, isOutput=False)
    bv_d = nc.declare_dram_parameter("bv", [1, G], F16, isOutput=False)
    sel8_d = nc.declare_dram_parameter("sel8", [NT, 128, 8], F16, isOutput=False)
    sel2_d = nc.declare_dram_parameter("sel2", [128, 2], F16, isOutput=False)
    sel8T_d = nc.declare_dram_parameter("sel8T", [8, NT, 128], F32R, isOutput=False)
    lnls_d = nc.declare_dram_parameter("lnls", [2, NT], F32, isOutput=False)
    lsbias_d = nc.declare_dram_parameter("lsbias", [128, 8], F32, isOutput=False)
    ident_d = nc.declare_dram_parameter("ident", [128, 128], F32R, isOutput=False)
    ones1_d = nc.declare_dram_parameter("ones1", [1, 128], F16, isOutput=False)
    out_d = nc.declare_dram_parameter("out", [N, C], F16, isOutput=True)

    qt_r = qt_d[:].rearrange("(cc p) n -> cc p n", p=128)
    kt_r = kt_d[:].rearrange("(cc p) n -> cc p n", p=128)
    vt_r = vt_d[:].rearrange("(cc p) n -> cc p n", p=128)
    wq_r = wq_d[:].rearrange("(g cc p) o -> g p cc o", g=2, p=128)
    wk_r = wk_d[:].rearrange("(g cc p) o -> g p cc o", g=2, p=128)
    wv_r = wv_d[:].rearrange("(g cc p) o -> g p cc o", g=2, p=128)

    # pre-load the ln+exp activation table once; every ACT op in this kernel
    # (Ln, Exp, Copy) is servable from it, so the auto-inserted loads (which
    # thrash between exp-only and ln-only tables) are avoided.
    table_names = list(get_activation_tables(nc.m.arch).keys())
    lnexp_id = table_names.index("natural_log_exp_and_others")

    with tile.TileContext(nc) as tc:
        nc.scalar.add_instruction(
            mybir.InstLoadActFuncSet(
                name=nc.get_next_instruction_name(), ins=[], outs=[],
                act_func_set_id=lnexp_id,
            )
        )
        with (
            tc.tile_pool(name="consts", bufs=1) as consts,
            tc.tile_pool(name="wo_p", bufs=1) as wo_p,
            tc.tile_pool(name="w_p", bufs=3) as w_p,
            tc.tile_pool(name="acts", bufs=16) as acts,
            tc.tile_pool(name="big", bufs=1) as big,
            tc.tile_pool(name="sq_p", bufs=2) as sq_p,
            tc.tile_pool(name="stats", bufs=1) as stats,
            tc.tile_pool(name="lssq_p", bufs=2) as lssq_p,
            tc.tile_pool(name="eT_p", bufs=26) as eT_p,
            tc.tile_pool(name="x_p", bufs=3) as x_p,
            tc.tile_pool(name="den_p", bufs=2) as den_p,
            tc.tile_pool(name="oB_p", bufs=5) as oB_p,
        ):
            # ---- persistent tiles ----
            qT = [big.tile([128, N], F16, tag=f"qT{t}", name=f"qT{t}") for t in range(NT)]
            kT = [big.tile([128, N], F16, tag=f"kT{t}", name=f"kT{t}") for t in range(NT)]
            v_sb = [big.tile([128, 8, HD + 1], F16, tag=f"v{s}", name=f"v{s}") for s in range(ST)]
            xt = [big.tile([128, N], F16, tag=f"xt{t}", name=f"xt{t}") for t in range(NT)]
            rskT = stats.tile([128, ST, 8], F32)
            rsq = stats.tile([8, N], F32R)
            rsk_t = [
                stats.tile([2, N], F32R, tag=f"rsk{t}", name=f"rsk{t}")
                for t in range(NT)
            ]
            # ones column of v (softmax denominator); SBUF-only so GPSIMD ok.
            for s in range(ST):
                nc.gpsimd.memset(v_sb[s][:, :, HD], 1.0)

            # ---- DMA stream (single SP queue; issue order = transfer order) ----
            # q stream first: the first projection wave starts ~2us in.
            wq_sb = w_p.tile([128, CC, G], F16, tag="w", name="wq")
            qch = []
            a0 = acts.tile([128, N], F16, tag="act", name="qt0")
            nc.sync.dma_start(out=a0[:], in_=qt_r[0])
            qch.append(a0)
            nc.sync.dma_start(out=wq_sb[:, 0:4, :], in_=wq_r[0])
            for cc in range(1, 4):
                a = acts.tile([128, N], F16, tag="act", name=f"qt{cc}")
                nc.sync.dma_start(out=a[:], in_=qt_r[cc])
                qch.append(a)
            nc.sync.dma_start(out=wq_sb[:, 4:8, :], in_=wq_r[1])
            for cc in range(4, 8):
                a = acts.tile([128, N], F16, tag="act", name=f"qt{cc}")
                nc.sync.dma_start(out=a[:], in_=qt_r[cc])
                qch.append(a)

            # consts needed during the q projection
            sel8 = consts.tile([128, NT, 8], F16)
            nc.sync.dma_start(out=sel8[:], in_=sel8_d[:].rearrange("t p e -> p t e"))
            sel2 = consts.tile([128, 2], F16)
            nc.sync.dma_start(out=sel2[:], in_=sel2_d[:])
            bq_sb = consts.tile([128, NT], F32)
            nc.sync.dma_start(out=bq_sb[:], in_=bq_d[:])

            wk_sb = w_p.tile([128, CC, G], F16, tag="w", name="wk")
            nc.sync.dma_start(out=wk_sb[:, 0:4, :], in_=wk_r[0])
            nc.sync.dma_start(out=wk_sb[:, 4:8, :], in_=wk_r[1])
            kch = []
            for cc in range(CC):
                a = acts.tile([128, N], F16, tag="act", name=f"kt{cc}")
                nc.sync.dma_start(out=a[:], in_=kt_r[cc])
                kch.append(a)

            sel8T = consts.tile([8, NT, 128], F32R)
            nc.sync.dma_start(out=sel8T[:], in_=sel8T_d[:])
            bk_sb = consts.tile([128, NT], F32)
            nc.sync.dma_start(out=bk_sb[:], in_=bk_d[:])
            lnls = consts.tile([2, NT], F32)
            nc.sync.dma_start(out=lnls[:], in_=lnls_d[:])
            ident = consts.tile([128, 128], F32R)
            nc.sync.dma_start(out=ident[:], in_=ident_d[:])
            lsbias = consts.tile([128, 8], F32)
            nc.sync.dma_start(out=lsbias[:], in_=lsbias_d[:])

            wv_sb = w_p.tile([128, CC, G], F16, tag="w", name="wv")
            nc.sync.dma_start(out=wv_sb[:, 0:4, :], in_=wv_r[0])
            nc.sync.dma_start(out=wv_sb[:, 4:8, :], in_=wv_r[1])
            vch = []
            for cc in range(CC):
                a = acts.tile([128, N], F16, tag="act", name=f"vt{cc}")
                nc.sync.dma_start(out=a[:], in_=vt_r[cc])
                vch.append(a)

            ones1 = consts.tile([1, 128], F16)
            nc.sync.dma_start(out=ones1[:], in_=ones1_d[:])
            bv_sb = consts.tile([1, G], F16)
            nc.sync.dma_start(out=bv_sb[:], in_=bv_d[:])
            wo_sb = wo_p.tile([128, NT, C], F16)
            nc.sync.dma_start(
                out=wo_sb[:], in_=wo_d[:].rearrange("(t p) c -> p t c", p=128)
            )

            # ---- phase 1: q projection (full) + k projection tile 0 ----
            deferred = deque()

            def flush(n=99):
                for _ in range(min(n, len(deferred))):
                    deferred.popleft()()

            with (
                tc.tile_pool(name="pp", bufs=4, space="PSUM") as pp,
                tc.tile_pool(name="pssq", bufs=2, space="PSUM") as pssq_p,
                tc.tile_pool(name="pbc", bufs=2, space="PSUM") as pbc,
            ):
                # q projection in 4 pair-tile waves; global per-head ssq.
                def q_wave(qc, tp, ssq_half):
                    pst = [
                        pp.tile([128, G], F32, tag="proj", name=f"qp{qc}{tp}{i}")
                        for i in range(2)
                    ]
                    for cc in range(CC):
                        for ti in range(2):
                            t = 2 * tp + ti
                            nc.tensor.matmul(
                                pst[ti][:],
                                wq_sb[:, cc, t * 128 : (t + 1) * 128],
                                qch[cc][:, qc * 512 : (qc + 1) * 512],
                                start=(cc == 0),
                                stop=(cc == CC - 1),
                            )
                    for ti in range(2):
                        t = 2 * tp + ti
                        nc.vector.tensor_scalar_add(
                            out=qT[t][:, qc * 512 : (qc + 1) * 512],
                            in0=pst[ti][:],
                            scalar1=bq_sb[:, t : t + 1],
                        )
                        sq = sq_p.tile([128, G], F16, tag="sq", name=f"sq{t}{qc}")
                        nc.vector.tensor_mul(
                            out=sq[:],
                            in0=qT[t][:, qc * 512 : (qc + 1) * 512],
                            in1=qT[t][:, qc * 512 : (qc + 1) * 512],
                        )

                        def ssq_mm(t=t, sq=sq):
                            nc.tensor.matmul(
                                ssq_half[:],
                                sel8[:, t, :],
                                sq[:],
                                start=(t == 0),
                                stop=(t == NT - 1),
                            )

                        deferred.append(ssq_mm)

                def q_norm_thunk(ssq_half, qc):
                    def run():
                        lssq = lssq_p.tile([8, G], F32, tag="lssq", name=f"lsq{qc}")
                        nc.scalar.activation(out=lssq[:], in_=ssq_half[:], func=AF.Ln)
                        nc.scalar.activation(
                            out=rsq[:, qc * 512 : (qc + 1) * 512],
                            in_=lssq[:], func=AF.Exp, scale=-0.5,
                        )

                    return run

                def bc_thunk(t, qc):
                    def run():
                        pb = pbc.tile([128, G], F32, tag="bc", name=f"bc{t}{qc}")
                        nc.tensor.matmul(
                            pb[:],
                            sel8T[:, t, :],
                            rsq[:, qc * 512 : (qc + 1) * 512],
                            start=True,
                            stop=True,
                        )
                        nc.vector.tensor_mul(
                            out=qT[t][:, qc * 512 : (qc + 1) * 512],
                            in0=qT[t][:, qc * 512 : (qc + 1) * 512],
                            in1=pb[:],
                        )

                    return run

                for qc in range(QC):
                    ssq_half = pssq_p.tile([8, G], F32, tag="ssq", name=f"sq_q{qc}")
                    for tp in range(2):
                        q_wave(qc, tp, ssq_half)
                        flush(2)
                    flush()
                    deferred.append(q_norm_thunk(ssq_half, qc))
                    for t in range(NT):
                        deferred.append(bc_thunk(t, qc))

                # k projection: single-tile waves.  Tile 0 runs here (prefix);
                # tiles 1..3 run as fillers inside the attention loop using the
                # phase-2 "po" pool.  Per-(tile, half) ssq + norm chains.
                def k_wave_half(t, qc, ps, half):
                    for cc in range(4 * half, 4 * half + 4):
                        nc.tensor.matmul(
                            ps[:],
                            wk_sb[:, cc, t * 128 : (t + 1) * 128],
                            kch[cc][:, qc * 512 : (qc + 1) * 512],
                            start=(cc == 0),
                            stop=(cc == CC - 1),
                        )

                def k_wave_finish(t, qc, ps):
                    nc.vector.tensor_scalar_add(
                        out=kT[t][:, qc * 512 : (qc + 1) * 512],
                        in0=ps[:],
                        scalar1=bk_sb[:, t : t + 1],
                    )
                    sq = sq_p.tile([128, G], F16, tag="sq", name=f"ksq{t}{qc}")
                    nc.vector.tensor_mul(
                        out=sq[:],
                        in0=kT[t][:, qc * 512 : (qc + 1) * 512],
                        in1=kT[t][:, qc * 512 : (qc + 1) * 512],
                    )
                    return sq

                def k_wave(t, qc, pool, ptag):
                    ps = pool.tile([128, G], F32, tag=ptag, name=f"kp{t}{qc}")
                    k_wave_half(t, qc, ps, 0)
                    k_wave_half(t, qc, ps, 1)
                    return k_wave_finish(t, qc, ps)

                def k_chain(t, qc, sq, pool, ptag):
                    # ssq (2 rows) -> sbuf bounce -> ln -> exp into rsk_t[t].
                    # The DVE bounce frees the PSUM tile immediately instead of
                    # holding it until ACT's (busy, in-order) queue reaches Ln.
                    ps = pool.tile([128, G], F32, tag=ptag, name=f"kn{t}{qc}")
                    nc.tensor.matmul(
                        ps[0:2, :], sel2[:], sq[:], start=True, stop=True
                    )
                    ssb = lssq_p.tile([2, G], F32, tag="ssb", name=f"skb{t}{qc}")
                    nc.vector.tensor_copy(out=ssb[:], in_=ps[0:2, :])
                    lssq = lssq_p.tile([2, G], F32, tag="lssq", name=f"lsk{t}{qc}")
                    nc.scalar.activation(out=lssq[:], in_=ssb[:], func=AF.Ln)
                    nc.scalar.activation(
                        out=rsk_t[t][:, qc * 512 : (qc + 1) * 512],
                        in_=lssq[:], func=AF.Exp, scale=-0.5,
                        bias=lnls[:, t : t + 1],
                    )

                def k_rskT(t, qc, pool, ptag):
                    # transpose rsk[2t:2t+2, half] into rskT[:, s, 2t:2t+2]
                    # for the 4 key tiles s of this half, all in one psum tile.
                    pt = pool.tile([128, G], F32, tag=ptag, name=f"krt{t}{qc}")
                    for i in range(4):
                        s = 4 * qc + i
                        nc.tensor.matmul(
                            pt[:].bitcast(F32R)[:, 2 * i : 2 * i + 2],
                            rsk_t[t][:, s * 128 : (s + 1) * 128],
                            ident[0:2, 0:2],
                            is_transpose=True,
                            start=(i == 0),
                            stop=(i == 3),
                        )
                    nc.vector.tensor_copy(
                        out=rskT[:, 4 * qc : 4 * qc + 4, 2 * t : 2 * t + 2],
                        in_=pt[:, 0:8].rearrange("p (s e) -> p s e", e=2),
                    )

                for t in range(2):
                    for qc in range(QC):
                        sq = k_wave(t, qc, pp, "proj")
                        flush(2)
                        k_chain(t, qc, sq, pbc, "bc")
                        k_rskT(t, qc, pbc, "bc")
                flush()

            # ---- phase 2: attention + v proj + k t1..3 + out-projection ----
            with (
                tc.tile_pool(name="psT", bufs=2, space="PSUM") as psT_p,
                tc.tile_pool(name="pv", bufs=1, space="PSUM") as pv_p,
                tc.tile_pool(name="po", bufs=2, space="PSUM") as po_p,
            ):
                ets = {}
                cur_pv = {}
                cur_x = [None] * 8
                vp_issued = [False] * ST
                vp_step = [99] * ST
                cur_g = [0]
                fill_hi = deque()
                fill_lo = deque()

                def sc(h, s):
                    t, j = divmod(h, 2)
                    st = psT_p.tile([128, N], F32, tag="sT", name=f"sT{h}_{s}")
                    for qc in range(QC):
                        nc.tensor.matmul(
                            st[:, qc * 512 : (qc + 1) * 512],
                            kT[t][j * 64 : (j + 1) * 64, s * 128 : (s + 1) * 128],
                            qT[t][j * 64 : (j + 1) * 64, qc * 512 : (qc + 1) * 512],
                            start=True,
                            stop=True,
                        )
                    e = eT_p.tile([128, N], F16, tag="eT", name=f"eT{h}_{s}")
                    nc.scalar.activation(
                        out=e[:], in_=st[:], func=AF.Exp,
                        bias=lsbias[:, h : h + 1],
                        scale=rskT[:, s, h : h + 1],
                    )
                    ets[(h, s)] = e

                vp_state = {}

                def vp_half(s, half):
                    if half == 0:
                        ps = po_p.tile([128, G], F32, tag="po", name=f"vp{s}")
                        vp_state[s] = ps
                    else:
                        ps = vp_state.pop(s)
                    for cc in range(4 * half, 4 * half + 4):
                        nc.tensor.matmul(
                            ps[:],
                            vch[cc][:, s * 128 : (s + 1) * 128],
                            wv_sb[:, cc, :],
                            start=(cc == 0),
                            stop=(not vbias_nonzero and cc == CC - 1),
                        )
                    if half == 1:
                        if vbias_nonzero:
                            nc.tensor.matmul(
                                ps[:], ones1[:], bv_sb[:], start=False, stop=True
                            )
                        nc.vector.tensor_copy(
                            out=v_sb[s][:, :, 0:HD],
                            in_=ps[:].rearrange("p (h d) -> p h d", h=8),
                        )
                        vp_issued[s] = True
                        vp_step[s] = cur_g[0]

                def pv_op(h, s):
                    pvA, pvB = cur_pv[h]
                    e = ets.pop((h, s))
                    for qb in range(8):
                        grp = pvA if qb < 4 else pvB
                        # one accumulation group per psum bank: the first
                        # matmul of the head starts (zeroing the region), the
                        # last stops
                        nc.tensor.matmul(
                            grp[:, qb % 4, :],
                            e[:, qb * 128 : (qb + 1) * 128],
                            v_sb[s][:, h, :],
                            start=(s == 0 and qb % 4 == 0),
                            stop=(s == ST - 1 and qb % 4 == 3),
                        )

                def transp(t, half, xx):
                    # four query-block transposes packed into one psum tile,
                    # copied out with a single wide op
                    pt = po_p.tile([128, G], F32, tag="po", name=f"tp{t}_{half}")
                    for i in range(4):
                        qb = 4 * half + i
                        nc.tensor.matmul(
                            pt[:].bitcast(F32R)[:, i * 128 : (i + 1) * 128],
                            xx[:, qb, :],
                            ident[:],
                            is_transpose=True,
                            start=(i == 0),
                            stop=(i == 3),
                        )
                    if t == NT - 1:
                        # tail: ACT is idle once the exps are done
                        nc.scalar.activation(
                            out=xt[t][:, half * 512 : (half + 1) * 512],
                            in_=pt[:], func=AF.Copy,
                        )
                    else:
                        nc.vector.tensor_copy(
                            out=xt[t][:, half * 512 : (half + 1) * 512], in_=pt[:]
                        )

                def head_end(h):
                    t, j = divmod(h, 2)
                    pvA, pvB = cur_pv.pop(h)
                    den = den_p.tile([128, 8], F32, tag="den", name=f"den{h}")
                    nc.vector.tensor_copy(out=den[:, 0:4], in_=pvA[:, :, HD])
                    nc.vector.tensor_copy(out=den[:, 4:8], in_=pvB[:, :, HD])
                    rden = den_p.tile([128, 8], F32, tag="rden", name=f"rden{h}")
                    nc.vector.reciprocal(out=rden[:], in_=den[:])
                    if j == 0:
                        cur_x[0] = x_p.tile(
                            [128, 8, 128], F32R, tag="xall", name=f"xall{t}"
                        )
                    for qb in range(8):
                        grp = pvA if qb < 4 else pvB
                        nc.vector.tensor_scalar_mul(
                            out=cur_x[0][:, qb, j * 64 : (j + 1) * 64],
                            in0=grp[:, qb % 4, 0:HD],
                            scalar1=rden[:, qb : qb + 1],
                        )
                    if j == 1:
                        for half in range(2):
                            fill_hi.append(
                                (700, 999, lambda t=t, half=half, xx=cur_x[0]: transp(t, half, xx))
                            )

                # k tiles 1..3 as fillers (phase-2 po pool), split into three
                # light thunks so no single step swallows a full wave.
                kf_state = {}

                def kf_a(t, qc):
                    def run():
                        ps = po_p.tile([128, G], F32, tag="po", name=f"kp{t}{qc}")
                        kf_state[(t, qc)] = ps
                        k_wave_half(t, qc, ps, 0)

                    return run

                def kf_b(t, qc):
                    def run():
                        ps = kf_state.pop((t, qc))
                        k_wave_half(t, qc, ps, 1)
                        sq = k_wave_finish(t, qc, ps)
                        kf_state[(t, qc, "sq")] = sq

                    return run

                def kf_c(t, qc):
                    def run():
                        sq = kf_state.pop((t, qc, "sq"))
                        k_chain(t, qc, sq, po_p, "po")
                        k_rskT(t, qc, po_p, "po")

                    return run

                # Filler queue with rough PE-cost credits (ns).  Order
                # respects deadlines: k-t1 by step 16, k-t2 by 32, k-t3 by 48;
                # v-proj gates only the (elastic) pv drain via vp_issued.
                def k_due(t, qc):
                    # consumed by sc(2t, s) at step 16t (+4 for the qc1 half)
                    return 16 * t + 4 * qc - 8

                for qc in range(QC):
                    fill_hi.append((900, k_due(2, qc), kf_a(2, qc)))
                    fill_hi.append((900, k_due(2, qc), kf_b(2, qc)))
                    fill_hi.append((500, k_due(2, qc), kf_c(2, qc)))
                for s in range(ST):
                    fill_hi.append((1000, 23 + s, lambda s=s: vp_half(s, 0)))
                    fill_hi.append((1000, 24 + s, lambda s=s: vp_half(s, 1)))
                for qc in range(QC):
                    fill_hi.append((900, k_due(3, qc), kf_a(3, qc)))
                    fill_hi.append((900, k_due(3, qc), kf_b(3, qc)))
                    fill_hi.append((500, k_due(3, qc), kf_c(3, qc)))

                pv_ptr = 0

                def pv_ready(ptr, g):
                    h_, s_ = divmod(ptr, 8)
                    if h_ == 0:
                        # wait ~2 steps past the v-projection wave so the pv
                        # matmul never parks at the PE queue head
                        return vp_issued[s_] and g >= vp_step[s_] + 2
                    lag = 3 if s_ == 0 else 2
                    return g >= 8 * h_ + s_ + lag

                def drain_pv(g, budget=2):
                    nonlocal pv_ptr
                    while budget > 0 and pv_ptr < 64 and pv_ready(pv_ptr, g):
                        h_, s_ = divmod(pv_ptr, 8)
                        if s_ == 0:
                            cur_pv[h_] = (
                                pv_p.tile([128, 4, HD + 1], F32, tag="pvA", name=f"pvA{h_}"),
                                pv_p.tile([128, 4, HD + 1], F32, tag="pvB", name=f"pvB{h_}"),
                            )
                        pv_op(h_, s_)
                        pv_ptr += 1
                        if s_ == ST - 1:
                            head_end(h_)
                        budget -= 1

                credit = 0.0
                RATION = 320.0  # ns of filler work per step
                for g in range(64):
                    h, s = divmod(g, 8)
                    cur_g[0] = g
                    credit += RATION
                    while fill_hi and (credit >= fill_hi[0][0] or fill_hi[0][1] <= g):
                        cost, _due, thunk = fill_hi.popleft()
                        credit = max(credit - cost, 0.0)
                        thunk()
                    sc(h, s)
                    backlog = 8 * h + s - pv_ptr
                    drain_pv(g, budget=3 if backlog > 12 else 2)

                # drain: remaining pv ops, last pair's transposes, out-proj.
                while pv_ptr < 64:
                    drain_pv(99, budget=8)
                while fill_hi:
                    fill_hi.popleft()[2]()
                # out-projection: 16 passes pipelined over 4 psum slots
                # (two halves of a psT tile + two po tiles) so the PE never
                # waits on the copy-out of the previous pass.
                for s_ in range(ST):
                    if s_ % 2 == 0:
                        stile = psT_p.tile([128, N], F32, tag="sT", name=f"ob{s_}")
                    ob = oB_p.tile([128, 2, G], F16, tag="oB", name=f"oB{s_}")
                    for coc in range(2):
                        if s_ % 2 == 0:
                            ps = stile[:, coc * 512 : (coc + 1) * 512]
                        else:
                            pot = po_p.tile([128, G], F32, tag="po", name=f"obp{s_}_{coc}")
                            ps = pot[:]
                        for t_ in range(NT):
                            nc.tensor.matmul(
                                ps,
                                xt[t_][:, s_ * 128 : (s_ + 1) * 128],
                                wo_sb[:, t_, coc * 512 : (coc + 1) * 512],
                                start=(t_ == 0),
                                stop=(t_ == NT - 1),
                            )
                        eng = nc.vector if coc == 0 else nc.scalar
                        if coc == 0:
                            nc.vector.tensor_copy(out=ob[:, 0, :], in_=ps)
                        else:
                            nc.scalar.activation(
                                out=ob[:, 1, :], in_=ps, func=AF.Copy
                            )
                    for coc in range(2):
                        nc.sync.dma_start(
                            out=out_d[:][
                                s_ * 128 : (s_ + 1) * 128, coc * 512 : (coc + 1) * 512
                            ],
                            in_=ob[:, coc, :],
                        )

    nc.compile()
    return nc


def kernel(
    query, key, value, in_proj_w, in_proj_b, logit_scale, out_w, out_b, **kw
):
    global _CACHED_NC, _LAST_IN_MAPS
    query = np.asarray(query, dtype=np.float32)
    key = np.asarray(key, dtype=np.float32)
    value = np.asarray(value, dtype=np.float32)
    in_proj_w = np.asarray(in_proj_w, dtype=np.float32)
    in_proj_b = np.asarray(in_proj_b, dtype=np.float32)
    logit_scale = np.asarray(logit_scale, dtype=np.float32)
    out_w = np.asarray(out_w, dtype=np.float32)
    out_b = np.asarray(out_b, dtype=np.float32)

    ls = np.exp(np.minimum(logit_scale.reshape(H), LOGIT_SCALE_MAX))  # (16,)
    vbias_nonzero = bool(np.any(in_proj_b[2 * C :]))

    sel8 = np.zeros((NT, 128, 8), dtype=np.float16)
    sel8T = np.zeros((8, NT, 128), dtype=np.float32)
    for t in range(NT):
        for p in range(128):
            h = 2 * t + p // 64
            sel8[t, p, h] = 1.0
            sel8T[h, t, p] = 1.0
    sel2 = np.zeros((128, 2), dtype=np.float16)
    sel2[0:64, 0] = 1.0
    sel2[64:128, 1] = 1.0
    ident = np.eye(128, dtype=np.float32)

    in_maps = []
    for c in range(8):
        b, g = c // 2, c % 2
        dims = slice(g * G, (g + 1) * G)
        ls_c = ls[g * 8 : (g + 1) * 8]  # (8,)
        in_maps.append(
            {
                "qt": np.ascontiguousarray(query[:, b, :].T, dtype=np.float16),
                "kt": np.ascontiguousarray(key[:, b, :].T, dtype=np.float16),
                "vt": np.ascontiguousarray(value[:, b, :].T, dtype=np.float16),
                "wq": np.ascontiguousarray(in_proj_w[0 * C :, :][dims, :].T, dtype=np.float16),
                "wk": np.ascontiguousarray(in_proj_w[1 * C :, :][dims, :].T, dtype=np.float16),
                "wv": np.ascontiguousarray(in_proj_w[2 * C :, :][dims, :].T, dtype=np.float16),
                "wo": np.ascontiguousarray(out_w[:, dims].T, dtype=np.float16),
                "bq": np.ascontiguousarray(
                    in_proj_b[0 * C :][dims].reshape(NT, 128).T, dtype=np.float32
                ),
                "bk": np.ascontiguousarray(
                    in_proj_b[1 * C :][dims].reshape(NT, 128).T, dtype=np.float32
                ),
                "bv": np.ascontiguousarray(
                    in_proj_b[2 * C :][dims].reshape(1, G), dtype=np.float16
                ),
                "sel8": sel8,
                "sel2": sel2,
                "sel8T": sel8T,
                "lnls": np.log(ls_c).reshape(NT, 2).T.astype(np.float32).copy(),
                "lsbias": np.repeat(-ls_c.reshape(1, 8), 128, axis=0).astype(np.float32),
                "ident": ident,
                "ones1": np.ones((1, 128), dtype=np.float16),
            }
        )

    _LAST_IN_MAPS = in_maps
    if vbias_nonzero not in _CACHED_NC:
        _CACHED_NC[vbias_nonzero] = build_nc(vbias_nonzero)
    res = run_bass_kernel_spmd(
        _CACHED_NC[vbias_nonzero], in_maps, core_ids=list(range(8))
    )

    out = np.zeros((N, B, C), dtype=np.float32)
    for c in range(8):
        b = c // 2
        out[:, b, :] += res.results[c]["out"].astype(np.float32)
    out += out_b.reshape(1, 1, C)
    return out


# revision 36
# speedup vs baseline: 1.0060x; 1.0022x over previous
# Trainium2 Bass kernel for nn_CustomAttention (cosine-sim multi-head attention).
#
# Sharding over 8 cores: core c handles batch b = c//2 and head group
# g = c%2 (8 of 16 heads, 512 feature dims).  Each core computes its heads'
# q/k/v projections (Megatron column-parallel), cosine-sim attention, and a
# partial output projection (row-parallel over its 512 dims).  The host sums
# the two partial outputs per batch and adds out_b.
#
# All heavy matmuls run in fp16 (1 cycle/row on the PE at any output width),
# with fp32 PSUM accumulation.  Layout highlights:
#   qT/kT: (dims=512, seq=1024) as 4 tiles of (128, 1024); head dims on
#          partitions so the scores matmul contracts head_dim on partitions.
#   scores are computed transposed, sT[k, q]; the per-key ls/||k|| factor and
#   the -ls bias fold into the exp() activation as per-partition APs.
#   1/||q|| and ls/||k|| come from exp(-0.5*ln(ssq) [+ ln ls]) so the whole
#   kernel uses a single activation table (ln+exp) -- no table reloads.
#   p@v runs transposed: x[q, d] = eT[k, q-block]^T @ v[k, d|1]; the appended
#   ones-column of v gives the softmax denominator, applied as a per-partition
#   (per-query) scalar during the PSUM->SBUF copy.  x tiles are PE-transposed
#   back to (dims, seq) for the out-proj.
#   out-proj is split into two half-contractions: t0+t1 overlaps the attention
#   stream, t2+t3 runs at the tail (the A half folded into the PSUM
#   accumulation via an identity matmul; copy-out split between DVE and ACT).
#
# Scheduling: engine queues drain strictly in issue order, so issue order ==
# execution order.  The attention main loop is ACT-paced (64 wide exps); the
# PE stream weaves scores, p@v, the v projection, the k projection for tiles
# t1..t3 (only q and k-t0 are done up front), x transposes and the first
# out-proj half into the gaps as "fillers".  GPSIMD cannot touch PSUM, so all
# PSUM->SBUF traffic is on DVE (and ACT Copy in the tail).

import math
import sys
from collections import deque

import numpy as np

sys.path.insert(0, "/opt/trn_rl_repo")

import concourse.bass as bass
import concourse.tile as tile
from concourse import bacc, mybir
from concourse.bass_utils import run_bass_kernel_spmd
from concourse.hw_specs import get_activation_tables

N = 1024  # sequence length
B = 4  # batch
C = 1024  # channels
H = 16  # total heads
HD = 64  # head dim
G = 512  # dims per core (8 heads)
NT = 4  # (128, N) tiles of qT/kT per core
CC = 8  # contraction chunks of 128 over C
ST = 8  # seq tiles of 128
QC = 2  # seq chunks of 512
LOGIT_SCALE_MAX = math.log(1.0 / 0.01)

F32 = mybir.dt.float32
F32R = mybir.dt.float32r
F16 = mybir.dt.float16
AF = mybir.ActivationFunctionType

_CACHED_NC = {}
_LAST_IN_MAPS = None


def build_nc(vbias_nonzero):
    nc = bacc.Bacc("TRN2", target_bir_lowering=False)

    qt_d = nc.declare_dram_parameter("qt", [C, N], F16, isOutput=False)
    kt_d = nc.declare_dram_parameter("kt", [C, N], F16, isOutput=False)
    vt_d = nc.declare_dram_parameter("vt", [C, N], F16, isOutput=False)
    wq_d = nc.declare_dram_parameter("wq", [C, G], F16, isOutput=False)
    wk_d = nc.declare_dram_parameter("wk", [C, G], F16, isOutput=False)
    wv_d = nc.declare_dram_parameter("wv", [C, G], F16, isOutput=False)
    wo_d = nc.declare_dram_parameter("wo", [G, C], F16, isOutput=False)
    bq_d = nc.declare_dram_parameter("bq", [128, NT], F32, isOutput=False)
    bk_d = nc.declare_dram_parameter("bk", [128, NT], F32, isOutput=False)
    bv_d = nc.declare_dram_parameter("bv", [1, G], F16, isOutput=False)
    sel8_d = nc.declare_dram_parameter("sel8", [NT, 128, 8], F16, isOutput=False)
    sel2_d = nc.declare_dram_parameter("sel2", [128, 2], F16, isOutput=False)
    sel8T_d = nc.declare_dram_parameter("sel8T", [8, NT, 128], F32R, isOutput=False)
    lnls_d = nc.declare_dram_parameter("lnls", [2, NT], F32, isOutput=False)
    lsbias_d = nc.declare_dram_parameter("lsbias", [128, 8], F32, isOutput=False)
    ident_d = nc.declare_dram_parameter("ident", [128, 128], F32R, isOutput=False)
    ones1_d = nc.declare_dram_parameter("ones1", [1, 128], F16, isOutput=False)
    out_d = nc.declare_dram_parameter("out", [N, C], F16, isOutput=True)

    qt_r = qt_d[:].rearrange("(cc p) n -> cc p n", p=128)
    kt_r = kt_d[:].rearrange("(cc p) n -> cc p n", p=128)
    vt_r = vt_d[:].rearrange("(cc p) n -> cc p n", p=128)
    wq_r = wq_d[:].rearrange("(g cc p) o -> g p cc o", g=2, p=128)
    wk_r = wk_d[:].rearrange("(g cc p) o -> g p cc o", g=2, p=128)
    wv_r = wv_d[:].rearrange("(g cc p) o -> g p cc o", g=2, p=128)

    # pre-load the ln+exp activation table once; every ACT op in this kernel
    # (Ln, Exp, Copy) is servable from it, so the auto-inserted loads (which
    # thrash between exp-only and ln-only tables) are avoided.
    table_names = list(get_activation_tables(nc.m.arch).keys())
    lnexp_id = table_names.index("natural_log_exp_and_others")

    with tile.TileContext(nc) as tc:
        nc.scalar.add_instruction(
            mybir.InstLoadActFuncSet(
                name=nc.get_next_instruction_name(), ins=[], outs=[],
                act_func_set_id=lnexp_id,
            )
        )
        with (
            tc.tile_pool(name="consts", bufs=1) as consts,
            tc.tile_pool(name="wo_p", bufs=1) as wo_p,
            tc.tile_pool(name="w_p", bufs=3) as w_p,
            tc.tile_pool(name="acts", bufs=16) as acts,
            tc.tile_pool(name="big", bufs=1) as big,
            tc.tile_pool(name="sq_p", bufs=2) as sq_p,
            tc.tile_pool(name="stats", bufs=1) as stats,
            tc.tile_pool(name="lssq_p", bufs=2) as lssq_p,
            tc.tile_pool(name="eT_p", bufs=26) as eT_p,
            tc.tile_pool(name="x_p", bufs=3) as x_p,
            tc.tile_pool(name="den_p", bufs=2) as den_p,
            tc.tile_pool(name="oB_p", bufs=5) as oB_p,
        ):
            # ---- persistent tiles ----
            qT = [big.tile([128, N], F16, tag=f"qT{t}", name=f"qT{t}") for t in range(NT)]
            kT = [big.tile([128, N], F16, tag=f"kT{t}", name=f"kT{t}") for t in range(NT)]
            v_sb = [big.tile([128, 8, HD + 1], F16, tag=f"v{s}", name=f"v{s}") for s in range(ST)]
            xt = [big.tile([128, N], F16, tag=f"xt{t}", name=f"xt{t}") for t in range(NT)]
            rskT = stats.tile([128, ST, 8], F32)
            rsq = stats.tile([8, N], F32R)
            rsk_t = [
                stats.tile([2, N], F32R, tag=f"rsk{t}", name=f"rsk{t}")
                for t in range(NT)
            ]
            # ones column of v (softmax denominator); SBUF-only so GPSIMD ok.
            for s in range(ST):
                nc.gpsimd.memset(v_sb[s][:, :, HD], 1.0)

            # ---- DMA stream (single SP queue; issue order = transfer order) ----
            # q stream first: the first projection wave starts ~2us in.
            wq_sb = w_p.tile([128, CC, G], F16, tag="w", name="wq")
            qch = []
            a0 = acts.tile([128, N], F16, tag="act", name="qt0")
            nc.sync.dma_start(out=a0[:], in_=qt_r[0])
            qch.append(a0)
            nc.sync.dma_start(out=wq_sb[:, 0:4, :], in_=wq_r[0])
            for cc in range(1, 4):
                a = acts.tile([128, N], F16, tag="act", name=f"qt{cc}")
                nc.sync.dma_start(out=a[:], in_=qt_r[cc])
                qch.append(a)
            nc.sync.dma_start(out=wq_sb[:, 4:8, :], in_=wq_r[1])
            for cc in range(4, 8):
                a = acts.tile([128, N], F16, tag="act", name=f"qt{cc}")
                nc.sync.dma_start(out=a[:], in_=qt_r[cc])
                qch.append(a)

            # consts needed during the q projection
            sel8 = consts.tile([128, NT, 8], F16)
            nc.sync.dma_start(out=sel8[:], in_=sel8_d[:].rearrange("t p e -> p t e"))
            sel2 = consts.tile([128, 2], F16)
            nc.sync.dma_start(out=sel2[:], in_=sel2_d[:])
            bq_sb = consts.tile([128, NT], F32)
            nc.sync.dma_start(out=bq_sb[:], in_=bq_d[:])

            wk_sb = w_p.tile([128, CC, G], F16, tag="w", name="wk")
            nc.sync.dma_start(out=wk_sb[:, 0:4, :], in_=wk_r[0])
            nc.sync.dma_start(out=wk_sb[:, 4:8, :], in_=wk_r[1])
            kch = []
            for cc in range(CC):
                a = acts.tile([128, N], F16, tag="act", name=f"kt{cc}")
                nc.sync.dma_start(out=a[:], in_=kt_r[cc])
                kch.append(a)

            sel8T = consts.tile([8, NT, 128], F32R)
            nc.sync.dma_start(out=sel8T[:], in_=sel8T_d[:])
            bk_sb = consts.tile([128, NT], F32)
            nc.sync.dma_start(out=bk_sb[:], in_=bk_d[:])
            lnls = consts.tile([2, NT], F32)
            nc.sync.dma_start(out=lnls[:], in_=lnls_d[:])
            ident = consts.tile([128, 128], F32R)
            nc.sync.dma_start(out=ident[:], in_=ident_d[:])
            lsbias = consts.tile([128, 8], F32)
            nc.sync.dma_start(out=lsbias[:], in_=lsbias_d[:])

            wv_sb = w_p.tile([128, CC, G], F16, tag="w", name="wv")
            nc.sync.dma_start(out=wv_sb[:, 0:4, :], in_=wv_r[0])
            nc.sync.dma_start(out=wv_sb[:, 4:8, :], in_=wv_r[1])
            vch = []
            for cc in range(CC):
                a = acts.tile([128, N], F16, tag="act", name=f"vt{cc}")
                nc.sync.dma_start(out=a[:], in_=vt_r[cc])
                vch.append(a)

            ones1 = consts.tile([1, 128], F16)
            nc.sync.dma_start(out=ones1[:], in_=ones1_d[:])
            bv_sb = consts.tile([1, G], F16)
            nc.sync.dma_start(out=bv_sb[:], in_=bv_d[:])
            wo_sb = wo_p.tile([128, NT, C], F16)
            nc.sync.dma_start(
                out=wo_sb[:], in_=wo_d[:].rearrange("(t p) c -> p t c", p=128)
            )

            # ---- phase 1: q projection (full) + k projection tile 0 ----
            deferred = deque()

            def flush(n=99):
                for _ in range(min(n, len(deferred))):
                    deferred.popleft()()

            with (
                tc.tile_pool(name="pp", bufs=4, space="PSUM") as pp,
                tc.tile_pool(name="pssq", bufs=2, space="PSUM") as pssq_p,
                tc.tile_pool(name="pbc", bufs=2, space="PSUM") as pbc,
            ):
                # q projection in 4 pair-tile waves; global per-head ssq.
                def q_wave(qc, tp, ssq_half):
                    pst = [
                        pp.tile([128, G], F32, tag="proj", name=f"qp{qc}{tp}{i}")
                        for i in range(2)
                    ]
                    for cc in range(CC):
                        for ti in range(2):
                            t = 2 * tp + ti
                            nc.tensor.matmul(
                                pst[ti][:],
                                wq_sb[:, cc, t * 128 : (t + 1) * 128],
                                qch[cc][:, qc * 512 : (qc + 1) * 512],
                                start=(cc == 0),
                                stop=(cc == CC - 1),
                            )
                    for ti in range(2):
                        t = 2 * tp + ti
                        nc.vector.tensor_scalar_add(
                            out=qT[t][:, qc * 512 : (qc + 1) * 512],
                            in0=pst[ti][:],
                            scalar1=bq_sb[:, t : t + 1],
                        )
                        sq = sq_p.tile([128, G], F16, tag="sq", name=f"sq{t}{qc}")
                        nc.vector.tensor_mul(
                            out=sq[:],
                            in0=qT[t][:, qc * 512 : (qc + 1) * 512],
                            in1=qT[t][:, qc * 512 : (qc + 1) * 512],
                        )

                        def ssq_mm(t=t, sq=sq):
                            nc.tensor.matmul(
                                ssq_half[:],
                                sel8[:, t, :],
                                sq[:],
                                start=(t == 0),
                                stop=(t == NT - 1),
                            )

                        deferred.append(ssq_mm)

                def q_norm_thunk(ssq_half, qc):
                    def run():
                        lssq = lssq_p.tile([8, G], F32, tag="lssq", name=f"lsq{qc}")
                        nc.scalar.activation(out=lssq[:], in_=ssq_half[:], func=AF.Ln)
                        nc.scalar.activation(
                            out=rsq[:, qc * 512 : (qc + 1) * 512],
                            in_=lssq[:], func=AF.Exp, scale=-0.5,
                        )

                    return run

                def bc_thunk(t, qc):
                    def run():
                        pb = pbc.tile([128, G], F32, tag="bc", name=f"bc{t}{qc}")
                        nc.tensor.matmul(
                            pb[:],
                            sel8T[:, t, :],
                            rsq[:, qc * 512 : (qc + 1) * 512],
                            start=True,
                            stop=True,
                        )
                        nc.vector.tensor_mul(
                            out=qT[t][:, qc * 512 : (qc + 1) * 512],
                            in0=qT[t][:, qc * 512 : (qc + 1) * 512],
                            in1=pb[:],
                        )

                    return run

                for qc in range(QC):
                    ssq_half = pssq_p.tile([8, G], F32, tag="ssq", name=f"sq_q{qc}")
                    for tp in range(2):
                        q_wave(qc, tp, ssq_half)
                        flush(2)
                    flush()
                    deferred.append(q_norm_thunk(ssq_half, qc))
                    for t in range(NT):
                        deferred.append(bc_thunk(t, qc))

                # k projection: single-tile waves.  Tile 0 runs here (prefix);
                # tiles 1..3 run as fillers inside the attention loop using the
                # phase-2 "po" pool.  Per-(tile, half) ssq + norm chains.
                def k_wave_half(t, qc, ps, half):
                    for cc in range(4 * half, 4 * half + 4):
                        nc.tensor.matmul(
                            ps[:],
                            wk_sb[:, cc, t * 128 : (t + 1) * 128],
                            kch[cc][:, qc * 512 : (qc + 1) * 512],
                            start=(cc == 0),
                            stop=(cc == CC - 1),
                        )

                def k_wave_finish(t, qc, ps):
                    nc.vector.tensor_scalar_add(
                        out=kT[t][:, qc * 512 : (qc + 1) * 512],
                        in0=ps[:],
                        scalar1=bk_sb[:, t : t + 1],
                    )
                    sq = sq_p.tile([128, G], F16, tag="sq", name=f"ksq{t}{qc}")
                    nc.vector.tensor_mul(
                        out=sq[:],
                        in0=kT[t][:, qc * 512 : (qc + 1) * 512],
                        in1=kT[t][:, qc * 512 : (qc + 1) * 512],
                    )
                    return sq

                def k_wave(t, qc, pool, ptag):
                    ps = pool.tile([128, G], F32, tag=ptag, name=f"kp{t}{qc}")
                    k_wave_half(t, qc, ps, 0)
                    k_wave_half(t, qc, ps, 1)
                    return k_wave_finish(t, qc, ps)

                def k_chain(t, qc, sq, pool, ptag):
                    # ssq (2 rows) -> sbuf bounce -> ln -> exp into rsk_t[t].
                    # The DVE bounce frees the PSUM tile immediately instead of
                    # holding it until ACT's (busy, in-order) queue reaches Ln.
                    ps = pool.tile([128, G], F32, tag=ptag, name=f"kn{t}{qc}")
                    nc.tensor.matmul(
                        ps[0:2, :], sel2[:], sq[:], start=True, stop=True
                    )
                    ssb = lssq_p.tile([2, G], F32, tag="ssb", name=f"skb{t}{qc}")
                    nc.vector.tensor_copy(out=ssb[:], in_=ps[0:2, :])
                    lssq = lssq_p.tile([2, G], F32, tag="lssq", name=f"lsk{t}{qc}")
                    nc.scalar.activation(out=lssq[:], in_=ssb[:], func=AF.Ln)
                    nc.scalar.activation(
                        out=rsk_t[t][:, qc * 512 : (qc + 1) * 512],
                        in_=lssq[:], func=AF.Exp, scale=-0.5,
                        bias=lnls[:, t : t + 1],
                    )

                def k_rskT(t, qc, pool, ptag):
                    # transpose rsk[2t:2t+2, half] into rskT[:, s, 2t:2t+2]
                    # for the 4 key tiles s of this half, all in one psum tile.
                    pt = pool.tile([128, G], F32, tag=ptag, name=f"krt{t}{qc}")
                    for i in range(4):
                        s = 4 * qc + i
                        nc.tensor.matmul(
                            pt[:].bitcast(F32R)[:, 2 * i : 2 * i + 2],
                            rsk_t[t][:, s * 128 : (s + 1) * 128],
                            ident[0:2, 0:2],
                            is_transpose=True,
                            start=(i == 0),
                            stop=(i == 3),
                        )
                    nc.vector.tensor_copy(
                        out=rskT[:, 4 * qc : 4 * qc + 4, 2 * t : 2 * t + 2],
                        in_=pt[:, 0:8].rearrange("p (s e) -> p s e", e=2),
                    )

                for t in range(2):
                    for qc in range(QC):
                        sq = k_wave(t, qc, pp, "proj")
                        flush(2)
                        k_chain(t, qc, sq, pbc, "bc")
                        k_rskT(t, qc, pbc, "bc")
                flush()

            # ---- phase 2: attention + v proj + k t1..3 + out-projection ----
            with (
                tc.tile_pool(name="psT", bufs=2, space="PSUM") as psT_p,
                tc.tile_pool(name="pv", bufs=1, space="PSUM") as pv_p,
                tc.tile_pool(name="po", bufs=2, space="PSUM") as po_p,
            ):
                ets = {}
                cur_pv = {}
                cur_x = [None] * 8
                vp_issued = [False] * ST
                vp_step = [99] * ST
                cur_g = [0]
                fill_hi = deque()
                fill_lo = deque()

                def sc(h, s):
                    t, j = divmod(h, 2)
                    st = psT_p.tile([128, N], F32, tag="sT", name=f"sT{h}_{s}")
                    for qc in range(QC):
                        nc.tensor.matmul(
                            st[:, qc * 512 : (qc + 1) * 512],
                            kT[t][j * 64 : (j + 1) * 64, s * 128 : (s + 1) * 128],
                            qT[t][j * 64 : (j + 1) * 64, qc * 512 : (qc + 1) * 512],
                            start=True,
                            stop=True,
                        )
                    e = eT_p.tile([128, N], F16, tag="eT", name=f"eT{h}_{s}")
                    nc.scalar.activation(
                        out=e[:], in_=st[:], func=AF.Exp,
                        bias=lsbias[:, h : h + 1],
                        scale=rskT[:, s, h : h + 1],
                    )
                    ets[(h, s)] = e

                vp_state = {}

                def vp_half(s, half):
                    if half == 0:
                        ps = po_p.tile([128, G], F32, tag="po", name=f"vp{s}")
                        vp_state[s] = ps
                    else:
                        ps = vp_state.pop(s)
                    for cc in range(4 * half, 4 * half + 4):
                        nc.tensor.matmul(
                            ps[:],
                            vch[cc][:, s * 128 : (s + 1) * 128],
                            wv_sb[:, cc, :],
                            start=(cc == 0),
                            stop=(not vbias_nonzero and cc == CC - 1),
                        )
                    if half == 1:
                        if vbias_nonzero:
                            nc.tensor.matmul(
                                ps[:], ones1[:], bv_sb[:], start=False, stop=True
                            )
                        nc.vector.tensor_copy(
                            out=v_sb[s][:, :, 0:HD],
                            in_=ps[:].rearrange("p (h d) -> p h d", h=8),
                        )
                        vp_issued[s] = True
                        vp_step[s] = cur_g[0]

                def pv_op(h, s):
                    pvA, pvB = cur_pv[h]
                    e = ets.pop((h, s))
                    for qb in range(8):
                        grp = pvA if qb < 4 else pvB
                        # one accumulation group per psum bank: the first
                        # matmul of the head starts (zeroing the region), the
                        # last stops
                        nc.tensor.matmul(
                            grp[:, qb % 4, :],
                            e[:, qb * 128 : (qb + 1) * 128],
                            v_sb[s][:, h, :],
                            start=(s == 0 and qb % 4 == 0),
                            stop=(s == ST - 1 and qb % 4 == 3),
                        )

                def transp(t, half, xx):
                    # four query-block transposes packed into one psum tile,
                    # copied out with a single wide op
                    pt = po_p.tile([128, G], F32, tag="po", name=f"tp{t}_{half}")
                    for i in range(4):
                        qb = 4 * half + i
                        nc.tensor.matmul(
                            pt[:].bitcast(F32R)[:, i * 128 : (i + 1) * 128],
                            xx[:, qb, :],
                            ident[:],
                            is_transpose=True,
                            start=(i == 0),
                            stop=(i == 3),
                        )
                    if t == NT - 1:
                        # tail: ACT is idle once the exps are done
                        nc.scalar.activation(
                            out=xt[t][:, half * 512 : (half + 1) * 512],
                            in_=pt[:], func=AF.Copy,
                        )
                    else:
                        nc.vector.tensor_copy(
                            out=xt[t][:, half * 512 : (half + 1) * 512], in_=pt[:]
                        )

                def head_end(h):
                    t, j = divmod(h, 2)
                    pvA, pvB = cur_pv.pop(h)
                    den = den_p.tile([128, 8], F32, tag="den", name=f"den{h}")
                    nc.vector.tensor_copy(out=den[:, 0:4], in_=pvA[:, :, HD])
                    nc.vector.tensor_copy(out=den[:, 4:8], in_=pvB[:, :, HD])
                    rden = den_p.tile([128, 8], F32, tag="rden", name=f"rden{h}")
                    nc.vector.reciprocal(out=rden[:], in_=den[:])
                    if j == 0:
                        cur_x[0] = x_p.tile(
                            [128, 8, 128], F32R, tag="xall", name=f"xall{t}"
                        )
                    for qb in range(8):
                        grp = pvA if qb < 4 else pvB
                        nc.vector.tensor_scalar_mul(
                            out=cur_x[0][:, qb, j * 64 : (j + 1) * 64],
                            in0=grp[:, qb % 4, 0:HD],
                            scalar1=rden[:, qb : qb + 1],
                        )
                    if j == 1:
                        for half in range(2):
                            fill_hi.append(
                                (700, 999, lambda t=t, half=half, xx=cur_x[0]: transp(t, half, xx))
                            )

                # k tiles 1..3 as fillers (phase-2 po pool), split into three
                # light thunks so no single step swallows a full wave.
                kf_state = {}

                def kf_a(t, qc):
                    def run():
                        ps = po_p.tile([128, G], F32, tag="po", name=f"kp{t}{qc}")
                        kf_state[(t, qc)] = ps
                        k_wave_half(t, qc, ps, 0)

                    return run

                def kf_b(t, qc):
                    def run():
                        ps = kf_state.pop((t, qc))
                        k_wave_half(t, qc, ps, 1)
                        sq = k_wave_finish(t, qc, ps)
                        kf_state[(t, qc, "sq")] = sq

                    return run

                def kf_c(t, qc):
                    def run():
                        sq = kf_state.pop((t, qc, "sq"))
                        k_chain(t, qc, sq, po_p, "po")
                        k_rskT(t, qc, po_p, "po")

                    return run

                # Filler queue with rough PE-cost credits (ns).  Order
                # respects deadlines: k-t1 by step 16, k-t2 by 32, k-t3 by 48;
                # v-proj gates only the (elastic) pv drain via vp_issued.
                def k_due(t, qc):
                    # consumed by sc(2t, s) at step 16t (+4 for the qc1 half)
                    return 16 * t + 4 * qc - 8

                for qc in range(QC):
                    fill_hi.append((900, k_due(2, qc), kf_a(2, qc)))
                    fill_hi.append((900, k_due(2, qc), kf_b(2, qc)))
                    fill_hi.append((500, k_due(2, qc), kf_c(2, qc)))
                for s in range(ST):
                    fill_hi.append((1000, 23 + s, lambda s=s: vp_half(s, 0)))
                    fill_hi.append((1000, 24 + s, lambda s=s: vp_half(s, 1)))
                for qc in range(QC):
                    fill_hi.append((900, k_due(3, qc), kf_a(3, qc)))
                    fill_hi.append((900, k_due(3, qc), kf_b(3, qc)))
                    fill_hi.append((500, k_due(3, qc), kf_c(3, qc)))

                pv_ptr = 0

                def pv_ready(ptr, g):
                    h_, s_ = divmod(ptr, 8)
                    if h_ == 0:
                        # wait ~2 steps past the v-projection wave so the pv
                        # matmul never parks at the PE queue head
                        return vp_issued[s_] and g >= vp_step[s_] + 2
                    lag = 4 if s_ == 0 else 2
                    return g >= 8 * h_ + s_ + lag

                def drain_pv(g, budget=2):
                    nonlocal pv_ptr
                    while budget > 0 and pv_ptr < 64 and pv_ready(pv_ptr, g):
                        h_, s_ = divmod(pv_ptr, 8)
                        if s_ == 0:
                            cur_pv[h_] = (
                                pv_p.tile([128, 4, HD + 1], F32, tag="pvA", name=f"pvA{h_}"),
                                pv_p.tile([128, 4, HD + 1], F32, tag="pvB", name=f"pvB{h_}"),
                            )
                        pv_op(h_, s_)
                        pv_ptr += 1
                        if s_ == ST - 1:
                            head_end(h_)
                        budget -= 1

                credit = 0.0
                RATION = 320.0  # ns of filler work per step
                for g in range(64):
                    h, s = divmod(g, 8)
                    cur_g[0] = g
                    credit += RATION
                    while fill_hi and (credit >= fill_hi[0][0] or fill_hi[0][1] <= g):
                        cost, _due, thunk = fill_hi.popleft()
                        credit = max(credit - cost, 0.0)
                        thunk()
                    sc(h, s)
                    backlog = 8 * h + s - pv_ptr
                    drain_pv(g, budget=3 if backlog > 12 else 2)

                # drain: remaining pv ops, last pair's transposes, out-proj.
                while pv_ptr < 64:
                    drain_pv(99, budget=8)
                while fill_hi:
                    fill_hi.popleft()[2]()
                # out-projection: 16 passes pipelined over 4 psum slots
                # (two halves of a psT tile + two po tiles) so the PE never
                # waits on the copy-out of the previous pass.
                for s_ in range(ST):
                    if s_ % 2 == 0:
                        stile = psT_p.tile([128, N], F32, tag="sT", name=f"ob{s_}")
                    ob = oB_p.tile([128, 2, G], F16, tag="oB", name=f"oB{s_}")
                    for coc in range(2):
                        if s_ % 2 == 0:
                            ps = stile[:, coc * 512 : (coc + 1) * 512]
                        else:
                            pot = po_p.tile([128, G], F32, tag="po", name=f"obp{s_}_{coc}")
                            ps = pot[:]
                        for t_ in range(NT):
                            nc.tensor.matmul(
                                ps,
                                xt[t_][:, s_ * 128 : (s_ + 1) * 128],
                                wo_sb[:, t_, coc * 512 : (coc + 1) * 512],
                                start=(t_ == 0),
                                stop=(t_ == NT - 1),
                            )
                        eng = nc.vector if coc == 0 else nc.scalar
                        if coc == 0:
                            nc.vector.tensor_copy(out=ob[:, 0, :], in_=ps)
                        else:
                            nc.scalar.activation(
                                out=ob[:, 1, :], in_=ps, func=AF.Copy
                            )
                    for coc in range(2):
                        nc.sync.dma_start(
                            out=out_d[:][
                                s_ * 128 : (s_ + 1) * 128, coc * 512 : (coc + 1) * 512
                            ],
                            in_=ob[:, coc, :],
                        )

    nc.compile()
    return nc


def kernel(
    query, key, value, in_proj_w, in_proj_b, logit_scale, out_w, out_b, **kw
):
    global _CACHED_NC, _LAST_IN_MAPS
    query = np.asarray(query, dtype=np.float32)
    key = np.asarray(key, dtype=np.float32)
    value = np.asarray(value, dtype=np.float32)
    in_proj_w = np.asarray(in_proj_w, dtype=np.float32)
    in_proj_b = np.asarray(in_proj_b, dtype=np.float32)
    logit_scale = np.asarray(logit_scale, dtype=np.float32)
    out_w = np.asarray(out_w, dtype=np.float32)
    out_b = np.asarray(out_b, dtype=np.float32)

    ls = np.exp(np.minimum(logit_scale.reshape(H), LOGIT_SCALE_MAX))  # (16,)
    vbias_nonzero = bool(np.any(in_proj_b[2 * C :]))

    sel8 = np.zeros((NT, 128, 8), dtype=np.float16)
    sel8T = np.zeros((8, NT, 128), dtype=np.float32)
    for t in range(NT):
        for p in range(128):
            h = 2 * t + p // 64
            sel8[t, p, h] = 1.0
            sel8T[h, t, p] = 1.0
    sel2 = np.zeros((128, 2), dtype=np.float16)
    sel2[0:64, 0] = 1.0
    sel2[64:128, 1] = 1.0
    ident = np.eye(128, dtype=np.float32)

    in_maps = []
    for c in range(8):
        b, g = c // 2, c % 2
        dims = slice(g * G, (g + 1) * G)
        ls_c = ls[g * 8 : (g + 1) * 8]  # (8,)
        in_maps.append(
            {
                "qt": np.ascontiguousarray(query[:, b, :].T, dtype=np.float16),
                "kt": np.ascontiguousarray(key[:, b, :].T, dtype=np.float16),
                "vt": np.ascontiguousarray(value[:, b, :].T, dtype=np.float16),
                "wq": np.ascontiguousarray(in_proj_w[0 * C :, :][dims, :].T, dtype=np.float16),
                "wk": np.ascontiguousarray(in_proj_w[1 * C :, :][dims, :].T, dtype=np.float16),
                "wv": np.ascontiguousarray(in_proj_w[2 * C :, :][dims, :].T, dtype=np.float16),
                "wo": np.ascontiguousarray(out_w[:, dims].T, dtype=np.float16),
                "bq": np.ascontiguousarray(
                    in_proj_b[0 * C :][dims].reshape(NT, 128).T, dtype=np.float32
                ),
                "bk": np.ascontiguousarray(
                    in_proj_b[1 * C :][dims].reshape(NT, 128).T, dtype=np.float32
                ),
                "bv": np.ascontiguousarray(
                    in_proj_b[2 * C :][dims].reshape(1, G), dtype=np.float16
                ),
                "sel8": sel8,
                "sel2": sel2,
                "sel8T": sel8T,
                "lnls": np.log(ls_c).reshape(NT, 2).T.astype(np.float32).copy(),
                "lsbias": np.repeat(-ls_c.reshape(1, 8), 128, axis=0).astype(np.float32),
                "ident": ident,
                "ones1": np.ones((1, 128), dtype=np.float16),
            }
        )

    _LAST_IN_MAPS = in_maps
    if vbias_nonzero not in _CACHED_NC:
        _CACHED_NC[vbias_nonzero] = build_nc(vbias_nonzero)
    res = run_bass_kernel_spmd(
        _CACHED_NC[vbias_nonzero], in_maps, core_ids=list(range(8))
    )

    out = np.zeros((N, B, C), dtype=np.float32)
    for c in range(8):
        b = c // 2
        out[:, b, :] += res.results[c]["out"].astype(np.float32)
    out += out_b.reshape(1, 1, C)
    return out
